# revision 25
# baseline (speedup 1.0000x reference)
"""Expert-choice MoE kernel for 8 Trainium2 NeuronCores (expert-parallel).

Decomposition (core e handles expert e):
  - router logits x . emb_e computed as bf16 hi/lo split (exact to ~2^-18)
    on PE; top-8 token indices per batch row via DVE max8/max_index; token
    gather via indirect DMA.
  - all activations/weights in fp16 (same bytes/speed as bf16, 8x less
    rounding noise), except: w1 fully in fp8-e3m4 and the first 2048 of
    w2's columns in fp8-e3m4 (scale 96, folded into x2h and the er
    copy-out respectively).
  - weights pre-packed host-side into partition-major [128, chunks, cols]
    layouts and streamed in 6-chunk groups (one DMA + a long matmul burst
    per group) so the PE stays busy and the DMA queue runs near the
    360 GB/s cap.
  - sum_weights GEMM1 column-sharded (each core owns 1536 columns of
    sw_w1).
  - ONE collective: AllGather of [er_e (64x3072) || z-partial (64x8)] in
    fp16. Cross-core wire DMAs are starved while the weight stream
    saturates the DMA engines, so pipelined AllReduces are useless here;
    a single post-stream AllGather whose input is only ready at stream
    end wastes nothing. Each core then sums z partials, softmaxes, and
    combines ws = sum_e wv[:,e]*er_e on DVE.
  - classification head sharded: GEMM1 column-shard (384 cols of ch_w1),
    GEMM2 contraction-shard (384 rows of ch_w2); per-core (64,1000)
    partials are summed on the host (+ ch_b2).
"""

import numpy as np
import ml_dtypes

import concourse.bass as bass
from concourse import bacc
import concourse.mybir as mybir
import concourse.tile as tile
from concourse.bass import ts, ds
from concourse.bass_utils import run_bass_kernel_spmd
from concourse.masks import make_identity

B, N, D, E, K, C = 64, 32, 384, 8, 8, 1000
KD, ND = K * D, N * D          # 3072, 12288
P = 128
NTOK = B * N                   # 2048
SWC = ND // E                  # 1536 sum-weights columns per core
CH1C = KD // E                 # 384 head-GEMM1 columns per core
KCE = KD // P                  # 24 k-chunks, expert GEMMs
KCS = ND // P                  # 96 k-chunks, sum-weights GEMM1
KCH = SWC // P                 # 12 k-chunks, z GEMM
NCORES = 8

G = 12                         # k-chunks per weight DMA group
W2F8C = 2048                   # fp8 columns of w2 (rest fp16)
S1 = 96.0                      # w1 fp8 scale (folded into x2h)
S2 = 96.0                      # w2 fp8 scale (folded into er copy-out)
AGW = KD + E                   # AllGather payload width (er || z)

F32 = mybir.dt.float32
F16 = mybir.dt.float16
BF16 = mybir.dt.bfloat16
F8E3 = mybir.dt.float8e3
U32 = mybir.dt.uint32
GELU = mybir.ActivationFunctionType.Gelu
EXP = mybir.ActivationFunctionType.Exp
X_AX = mybir.AxisListType.X
ADD = mybir.AluOpType.add
BYPASS = mybir.AluOpType.bypass
f16 = np.float16
bf16 = ml_dtypes.bfloat16
e3m4 = ml_dtypes.float8_e3m4


def _build(include_bias: bool) -> bass.Bass:
    nc = bacc.Bacc("TRN2", num_devices=NCORES)

    # xt: x.T in fp32 (router must match the reference's fp32 top-8 exactly;
    # a bf16 hi/lo split was ~8e-6 off and flipped one near-tie), emb_e
    # appended as column 2048
    xt = nc.dram_tensor("xt", [P, 3, NTOK + 1], F32, kind="ExternalInput")
    x2h = nc.dram_tensor("x2h", [NTOK, D], F16, kind="ExternalInput")
    xft = nc.dram_tensor("xft", [P, KCS, B], F16, kind="ExternalInput")
    sw1p = nc.dram_tensor("sw1p", [P, KCS, SWC], F16, kind="ExternalInput")
    w1p = nc.dram_tensor("w1p", [P, KCE, KD], F8E3, kind="ExternalInput")
    w2p8 = nc.dram_tensor("w2p8", [P, KCE, W2F8C], F8E3, kind="ExternalInput")
    w2p16 = nc.dram_tensor(
        "w2p16", [P, KCE, KD - W2F8C], F16, kind="ExternalInput")
    sw2p = nc.dram_tensor("sw2p", [P, KCH, E], F16, kind="ExternalInput")
    ch1p = nc.dram_tensor("ch1p", [P, KCE, CH1C], F16, kind="ExternalInput")
    ch2p = nc.dram_tensor("ch2p", [P, 3, C], F16, kind="ExternalInput")
    if include_bias:
        b1d = nc.dram_tensor("b1d", [1, KD], F32, kind="ExternalInput")
        b2d = nc.dram_tensor("b2d", [1, KD], F32, kind="ExternalInput")
        swb1d = nc.dram_tensor("swb1d", [1, SWC], F32, kind="ExternalInput")
        swb2d = nc.dram_tensor("swb2d", [1, E], F32, kind="ExternalInput")
        chb1d = nc.dram_tensor("chb1d", [1, CH1C], F32, kind="ExternalInput")
    outp = nc.dram_tensor("outp", [B, C], F32, kind="ExternalOutput")

    with tile.TileContext(nc) as tc:
        with (
            tc.tile_pool(name="consts", bufs=1) as consts,
            tc.tile_pool(name="acts", bufs=1) as acts,
            tc.tile_pool(name="wpool", bufs=2) as wpool,
            tc.tile_pool(name="ergp", bufs=2) as ergp,
            tc.tile_pool(name="ps_mm", bufs=6, space="PSUM") as ps_mm,
            tc.tile_pool(name="ps_tr", bufs=2, space="PSUM") as ps_tr,
            tc.tile_pool(name="dram", bufs=1, space="DRAM") as dram,
        ):
            # ---- constants / persistent activations ----
            ident = consts.tile([P, P], F16)
            make_identity(nc, ident[:])
            xft_sb = consts.tile([P, KCS, B], F16)
            nc.sync.dma_start(xft_sb[:], xft[:])
            sw2_sb = consts.tile([P, KCH, E], F16)
            nc.sync.dma_start(sw2_sb[:], sw2p[:])
            pwarm = ps_tr.tile([P, B], F16, name="pwarm", tag="pt")
            nc.tensor.transpose(pwarm[:32, :32], ident[:32, :32], ident[:32, :32])
            if include_bias:
                b1_sb = consts.tile([B, KD], F32)
                nc.scalar.dma_start(b1_sb[:], b1d[0:1, :].to_broadcast([B, KD]))
                b2_sb = consts.tile([B, KD], F32)
                nc.scalar.dma_start(b2_sb[:], b2d[0:1, :].to_broadcast([B, KD]))
                swb1_sb = consts.tile([B, SWC], F32)
                nc.scalar.dma_start(swb1_sb[:], swb1d[0:1, :].to_broadcast([B, SWC]))
                swb2_sb = consts.tile([B, E], F32)
                nc.scalar.dma_start(swb2_sb[:], swb2d[0:1, :].to_broadcast([B, E]))
                chb1_sb = consts.tile([B, CH1C], F32)
                nc.scalar.dma_start(chb1_sb[:], chb1d[0:1, :].to_broadcast([B, CH1C]))

            # ---- phase 1: sum-weights GEMM1, 16 groups of 6 k-chunks ----
            pms = [ps_mm.tile([B, 512], F32, name=f"pms{n}", tag="pm")
                   for n in range(3)]

            def sw_group(g):
                wt = wpool.tile([P, G, SWC], F16, name="wt", tag="wt")
                nc.sync.dma_start(wt[:], sw1p[:, ts(g, G), :])
                for cg in range(G):
                    c = g * G + cg
                    for n in range(3):
                        nc.tensor.matmul(
                            pms[n][:], xft_sb[:, c, :], wt[:, cg, ts(n, 512)],
                            start=(c == 0), stop=(c == KCS - 1),
                        )

            # xt rides the scalar queue so the sw weight stream never pauses
            xt_sb = acts.tile([P, 3, NTOK + 1], F32)
            nc.scalar.dma_start(xt_sb[:], xt[:])
            sw_group(0)
            sw_group(1)

            # ---- router: logits = x @ emb_e, fp32 (emb packed as col 2048) ----
            lg_flat = acts.tile([1, NTOK], F32)
            for nt in range(4):
                pr = ps_mm.tile([B, 512], F32, name="pr", tag="pm")
                for c in range(3):
                    nc.tensor.matmul(
                        pr[:1, :], xt_sb[:, c, NTOK : NTOK + 1],
                        xt_sb[:, c, ts(nt, 512)],
                        start=(c == 0), stop=(c == 2),
                    )
                nc.vector.tensor_copy(lg_flat[:, ts(nt, 512)], pr[:1, :])
            lg_dram = dram.tile([1, NTOK], F32)
            nc.scalar.dma_start(lg_dram[:], lg_flat[:])
            lg_bn = acts.tile([B, N], F32)
            nc.scalar.dma_start(lg_bn[:], lg_dram[:].rearrange("x (b n) -> (x b) n", b=B))

            # ---- top-8 tokens per row + gather (overlaps sw streaming) ----
            vals8 = acts.tile([B, 8], F32)
            idx8 = acts.tile([B, 8], U32)
            nc.vector.max(out=vals8[:], in_=lg_bn[:])
            nc.vector.max_index(out=idx8[:], in_max=vals8[:], in_values=lg_bn[:])
            base = acts.tile([B, 1], U32)
            nc.gpsimd.iota(base[:], pattern=[[0, 1]], base=0, channel_multiplier=N)
            off = acts.tile([B, 8], U32)
            nc.vector.tensor_tensor(
                out=off[:], in0=idx8[:], in1=base[:].to_broadcast([B, 8]), op=ADD
            )
            sel = acts.tile([B, K, D], F16)
            for k in range(K):
                nc.gpsimd.indirect_dma_start(
                    out=sel[:, k, :], out_offset=None,
                    in_=x2h[:],
                    in_offset=bass.IndirectOffsetOnAxis(ap=off[:, k : k + 1], axis=0),
                )
            sel_flat = sel[:].rearrange("b k d -> b (k d)")

            # ---- rest of the sw stream; selT transposes sprinkled in ----
            selT = acts.tile([P, KCE, B], F16)
            for g in range(2, KCS // G):
                sw_group(g)
                if g >= 5:
                    for cc in range(8):
                        c = (g - 5) * 8 + cc
                        pt = ps_tr.tile([P, B], F16, name="pt", tag="pt")
                        nc.tensor.transpose(pt[:], sel_flat[:, ts(c, P)], ident[:B, :B])
                        nc.vector.tensor_copy(selT[:, c, :], pt[:])
            ch2_sb = consts.tile([P, 3, C], F16)
            nc.sync.dma_start(ch2_sb[:], ch2p[:])

            # ---- h1 = gelu(sw psums); z partial [64, 8] ----
            h1 = acts.tile([B, SWC], F16)
            for n in range(3):
                if include_bias:
                    nc.vector.tensor_add(pms[n][:], pms[n][:], swb1_sb[:, ts(n, 512)])
                nc.scalar.activation(h1[:, ts(n, 512)], pms[n][:], GELU)
            h1T = acts.tile([P, KCH, B], F16)
            for c in range(KCH):
                pt = ps_tr.tile([P, B], F16, name="pt", tag="pt")
                nc.tensor.transpose(pt[:], h1[:, ts(c, P)], ident[:B, :B])
                nc.vector.tensor_copy(h1T[:, c, :], pt[:])
            pz = ps_mm.tile([B, E], F32, name="pz", tag="pm")
            for c in range(KCH):
                nc.tensor.matmul(
                    pz[:], h1T[:, c, :], sw2_sb[:, c, :],
                    start=(c == 0), stop=(c == KCH - 1),
                )
            # z partials go in their own tiny early AllReduce; its wire DMAs
            # are starved while the weight stream runs, but it drains during
            # the stream's dips/end and is done before the big er AllGather
            # lands, so the softmax is off the critical path.
            z_sb = acts.tile([B, E], F32)
            nc.vector.tensor_copy(z_sb[:], pz[:])
            zin = dram.tile([B, E], F32)
            zout = dram.tile([B, E], F32)
            nc.gpsimd.dma_start(zin[:], z_sb[:])
            nc.gpsimd.collective_compute(
                "AllReduce", ADD, replica_groups=[list(range(NCORES))],
                ins=[zin[:].opt()], outs=[zout[:].opt()],
            )

            # ---- phase 2: expert GEMM1, fp8 w1 stream (4 groups of 6) ----
            h = acts.tile([B, KD], F16)
            pme = [ps_mm.tile([B, 512], F32, name=f"pme{n}", tag="pm")
                   for n in range(6)]
            for g in range(KCE // G):
                wt8 = wpool.tile([P, G, KD], F8E3, name="wt8", tag="wt")
                nc.sync.dma_start(wt8[:], w1p[:, ts(g, G), :])
                for cg in range(G):
                    c = g * G + cg
                    for n in range(6):
                        nc.tensor.matmul(
                            pme[n][:], selT[:, c, :], wt8[:, cg, ts(n, 512)],
                            start=(c == 0), stop=(c == KCE - 1),
                        )
            last_gelu = None
            for n in range(6):
                if include_bias:
                    nc.vector.tensor_add(pme[n][:], pme[n][:], b1_sb[:, ts(n, 512)])
                last_gelu = nc.scalar.activation(h[:, ts(n, 512)], pme[n][:], GELU)
            hT = acts.tile([P, KCE, B], F16)
            for c in range(KCE):
                pt = ps_tr.tile([P, B], F16, name="pt", tag="pt")
                nc.tensor.transpose(pt[:], h[:, ts(c, P)], ident[:B, :B])
                nc.vector.tensor_copy(hT[:, c, :], pt[:])

            # ---- phase 3: expert GEMM2, single pass, 6 psum banks ----
            # cols 0:2048 fp8 (scale S2 undone at copy-out), 2048:3072 fp16
            ch1_sb = acts.tile([P, KCE, CH1C], F16)
            pm2 = [ps_mm.tile([B, 512], F32, name=f"pm2{n}", tag="pm")
                   for n in range(6)]
            last_ermm = None
            for g in range(KCE // G):
                wt2a = wpool.tile([P, G, W2F8C], F8E3, name="wt2a", tag="wt")
                nc.sync.dma_start(wt2a[:], w2p8[:, ts(g, G), :])
                wt2b = wpool.tile([P, G, KD - W2F8C], F16, name="wt2b", tag="wt")
                nc.sync.dma_start(wt2b[:], w2p16[:, ts(g, G), :])
                for cg in range(G):
                    c = g * G + cg
                    for n in range(6):
                        if n < 4:
                            rhs = wt2a[:, cg, ts(n, 512)]
                        else:
                            rhs = wt2b[:, cg, ts(n - 4, 512)]
                        last_ermm = nc.tensor.matmul(
                            pm2[n][:], hT[:, c, :], rhs,
                            start=(c == 0), stop=(c == KCE - 1),
                        )
                # stream the head GEMM1 weights behind the w2 groups
                nc.sync.dma_start(ch1_sb[:, ts(g, G), :], ch1p[:, ts(g, G), :])
            # er copy-out (1/S2 undone on the fp8 columns)
            er_sb = acts.tile([B, KD], F16)
            last_erc = None
            for n in range(6):
                if include_bias:
                    stage = acts.tile([B, 512], F32, name=f"stage{n}")
                    sc = (1.0 / S2) if n < 4 else 1.0
                    nc.vector.tensor_scalar_mul(stage[:], pm2[n][:], sc)
                    nc.vector.tensor_add(stage[:], stage[:], b2_sb[:, ts(n, 512)])
                    last_erc = nc.vector.tensor_copy(er_sb[:, ts(n, 512)], stage[:])
                elif n < 4:
                    last_erc = nc.vector.tensor_scalar_mul(
                        er_sb[:, ts(n, 512)], pm2[n][:], 1.0 / S2)
                else:
                    last_erc = nc.vector.tensor_copy(er_sb[:, ts(n, 512)], pm2[n][:])

            # ---- the big collective: AllGather of er ----
            cin = dram.tile([B, KD], F16)
            gout = dram.tile([NCORES * B, KD], F16)
            cin_dma = nc.scalar.dma_start(cin[:], er_sb[:])
            nc.gpsimd.collective_compute(
                "AllGather", BYPASS, replica_groups=[list(range(NCORES))],
                ins=[cin[:].opt()], outs=[gout[:].opt()],
            )

            # ---- softmax over experts -> wv, broadcast to wvb ----
            zb = acts.tile([B, E], F32)
            zb_dma = nc.gpsimd.dma_start(zb[:], zout[:])
            tile.add_dep_helper(zb_dma.ins, cin_dma.ins, sync=False,
                                reason="zb read after AG send kickoff")
            if include_bias:
                nc.vector.tensor_add(zb[:], zb[:], swb2_sb[:])
            mx = acts.tile([B, 1], F32)
            mx_i = nc.vector.reduce_max(mx[:], zb[:], axis=X_AX)
            tile.add_dep_helper(mx_i.ins, last_erc.ins, sync=False,
                                reason="softmax after er transpose copies on DVE")
            nmx = acts.tile([B, 1], F32)
            nc.vector.tensor_scalar_mul(nmx[:], mx[:], -1.0)
            exps = acts.tile([B, E], F32)
            exp_i = nc.scalar.activation(exps[:], zb[:], EXP, bias=nmx[:])
            tile.add_dep_helper(exp_i.ins, last_gelu.ins, sync=False,
                                reason="Exp after expert gelus on ACT")
            sm = acts.tile([B, 1], F32)
            nc.vector.reduce_sum(sm[:], exps[:], axis=X_AX)
            rs = acts.tile([B, 1], F32)
            nc.vector.reciprocal(rs[:], sm[:])
            wv = acts.tile([B, E], F32)
            nc.vector.tensor_scalar_mul(wv[:], exps[:], rs[:])

            # ---- combine: ws = sum_e wv[:, e] * er_e (streamed, fp16) ----
            # [B, KD] layout so the per-expert weight is a per-partition
            # scalar (fast DVE tensor_scalar path).
            ws = acts.tile([B, KD], F16)
            t16 = acts.tile([B, KD], F16)
            rb_dmas = []
            for r in range(NCORES):
                erg = ergp.tile([B, KD], F16, name="erg", tag="erg")
                i = nc.scalar.dma_start(erg[:], gout[ds(r * B, B), :])
                rb_dmas.append(i)
                if r == 0:
                    nc.vector.tensor_scalar_mul(ws[:], erg[:], wv[:, 0:1])
                else:
                    nc.vector.tensor_scalar_mul(t16[:], erg[:], wv[:, r : r + 1])
                    nc.vector.tensor_add(ws[:], ws[:], t16[:])
            tile.add_dep_helper(rb_dmas[0].ins, cin_dma.ins, sync=False,
                                reason="readbacks after AG send on scalar q")

            # ---- head GEMM1 (column shard): hh = gelu(ws @ ch1_e) ----
            # transposes first (PE pipelines with the DVE copies), then the
            # matmul burst back-to-back — avoids 24 PE<->DVE sem round trips.
            wsT = acts.tile([P, KCE, B], F16)
            for c in range(KCE):
                pt = ps_tr.tile([P, B], F16, name="pt", tag="pt")
                tr_i = nc.tensor.transpose(pt[:], ws[:, ts(c, P)], ident[:B, :B])
                if c == 0:
                    tile.add_dep_helper(tr_i.ins, last_ermm.ins, sync=False,
                                        reason="head transposes after er matmuls on PE")
                nc.vector.tensor_copy(wsT[:, c, :], pt[:])
            pmh = ps_mm.tile([B, CH1C], F32, name="pmh", tag="pm")
            for c in range(KCE):
                nc.tensor.matmul(
                    pmh[:], wsT[:, c, :], ch1_sb[:, c, :],
                    start=(c == 0), stop=(c == KCE - 1),
                )
            if include_bias:
                nc.vector.tensor_add(pmh[:], pmh[:], chb1_sb[:])
            hh = acts.tile([B, CH1C], F16)
            nc.scalar.activation(hh[:], pmh[:], GELU)

            # ---- head GEMM2 (contraction shard): out_part = hh @ ch2_e ----
            hhT = acts.tile([P, 3, B], F16)
            for c in range(3):
                pt = ps_tr.tile([P, B], F16, name="pt", tag="pt")
                nc.tensor.transpose(pt[:], hh[:, ts(c, P)], ident[:B, :B])
                nc.vector.tensor_copy(hhT[:, c, :], pt[:])
            outsb = acts.tile([B, C], F32)
            for nn in range(2):
                pmo = ps_mm.tile([B, 512], F32, name="pmo", tag="pm")
                for c in range(3):
                    nc.tensor.matmul(
                        pmo[:, :500], hhT[:, c, :], ch2_sb[:, c, ds(nn * 500, 500)],
                        start=(c == 0), stop=(c == 2),
                    )
                nc.vector.tensor_copy(outsb[:, ds(nn * 500, 500)], pmo[:, :500])
            nc.sync.dma_start(outp[:], outsb[:])

    nc.finalize()
    return nc


_NC_CACHE: dict = {}


def _get_nc(include_bias: bool) -> bass.Bass:
    if include_bias not in _NC_CACHE:
        _NC_CACHE[include_bias] = _build(include_bias)
    return _NC_CACHE[include_bias]


def _pmajor(a, nchunk, dt):
    """[nchunk*128, cols] -> [128, nchunk, cols] partition-major, cast."""
    cols = a.shape[1]
    return np.ascontiguousarray(
        a.reshape(nchunk, P, cols).transpose(1, 0, 2)).astype(dt)


def _pack_inputs(inputs: dict, include_bias: bool) -> list[dict]:
    fp32 = np.float32
    x = np.ascontiguousarray(np.asarray(inputs["x"], dtype=fp32))  # (64,32,384)
    expert_emb = np.asarray(inputs["expert_emb"], dtype=fp32)      # (8,384)
    w1 = np.asarray(inputs["w1"], fp32)                            # (8,3072,3072)
    w2 = np.asarray(inputs["w2"], fp32)
    sw_w1 = np.asarray(inputs["sw_w1"], fp32)                      # (12288,12288)
    sw_w2 = np.asarray(inputs["sw_w2"], fp32)                      # (12288,8)
    ch_w1 = np.asarray(inputs["ch_w1"], fp32)                      # (3072,3072)
    ch_w2 = np.asarray(inputs["ch_w2"], fp32)                      # (3072,1000)

    x2 = x.reshape(NTOK, D)
    xt_base = x2.T.reshape(3, P, NTOK).transpose(1, 0, 2)          # (128,3,2048)
    x2h = (x2 / S1).astype(f16)                                    # (2048,384)
    xf = x.reshape(B, ND)
    xft_p = _pmajor(np.ascontiguousarray(xf.T), KCS, f16)          # (128,96,64)

    in_maps = []
    for e in range(NCORES):
        emb_p = expert_emb[e].reshape(3, P).T                      # (128,3)
        xt_p = np.ascontiguousarray(
            np.concatenate([xt_base, emb_p[:, :, None]], axis=2), dtype=fp32)
        sw1_e = np.ascontiguousarray(sw_w1[:, e * SWC:(e + 1) * SWC])
        sw1_p = _pmajor(sw1_e, KCS, f16)                           # (128,96,1536)
        w1_p = _pmajor(w1[e] * S1, KCE, e3m4)                      # (128,24,3072)
        w2p8 = _pmajor(w2[e][:, :W2F8C] * S2, KCE, e3m4)           # (128,24,2048)
        w2p16 = _pmajor(w2[e][:, W2F8C:], KCE, f16)                # (128,24,1024)
        sw2_e = np.ascontiguousarray(sw_w2[e * SWC:(e + 1) * SWC, :])
        sw2_p = _pmajor(sw2_e, KCH, f16)                           # (128,12,8)
        ch1_p = _pmajor(
            np.ascontiguousarray(ch_w1[:, e * CH1C:(e + 1) * CH1C]), KCE, f16)
        ch2_p = _pmajor(
            np.ascontiguousarray(ch_w2[e * CH1C:(e + 1) * CH1C, :]), 3, f16)
        m = {
            "xt": xt_p, "x2h": x2h, "xft": xft_p, "sw1p": sw1_p,
            "w1p": w1_p, "w2p8": w2p8, "w2p16": w2p16,
            "sw2p": sw2_p, "ch1p": ch1_p, "ch2p": ch2_p,
        }
        if include_bias:
            m["b1d"] = np.asarray(inputs["b1"][e], fp32).reshape(1, KD)
            m["b2d"] = np.asarray(inputs["b2"][e], fp32).reshape(1, KD)
            m["swb1d"] = np.asarray(
                inputs["sw_b1"], fp32).reshape(1, ND)[:, e * SWC:(e + 1) * SWC]
            m["swb2d"] = np.asarray(inputs["sw_b2"], fp32).reshape(1, E)
            m["chb1d"] = np.asarray(
                inputs["ch_b1"], fp32).reshape(1, KD)[:, e * CH1C:(e + 1) * CH1C]
        in_maps.append(m)
    return in_maps


def _need_bias(inputs) -> bool:
    return any(
        float(np.abs(np.asarray(inputs[k])).max()) != 0.0
        for k in ("b1", "b2", "sw_b1", "sw_b2", "ch_b1")
    )


def run(inputs: dict, **run_kwargs):
    """Run on the 8 cores; returns (full_output, BassKernelResults)."""
    include_bias = _need_bias(inputs)
    nc = _get_nc(include_bias)
    in_maps = _pack_inputs(inputs, include_bias)
    res = run_bass_kernel_spmd(nc, in_maps, core_ids=list(range(NCORES)), **run_kwargs)
    out = np.zeros((B, C), dtype=np.float64)
    for e in range(NCORES):
        out += res.results[e]["outp"].astype(np.float64)
    out += np.asarray(inputs["ch_b2"], np.float64)
    return out.astype(np.float32), res


def kernel(**inputs) -> np.ndarray:
    out, _ = run(inputs)
    return out


# revision 26
# speedup vs baseline: 1.0325x; 1.0325x over previous
"""Expert-choice MoE kernel for 8 Trainium2 NeuronCores (expert-parallel).

Decomposition (core e handles expert e):
  - router logits x . emb_e computed as bf16 hi/lo split (exact to ~2^-18)
    on PE; top-8 token indices per batch row via DVE max8/max_index; token
    gather via indirect DMA.
  - all activations/weights in fp16 (same bytes/speed as bf16, 8x less
    rounding noise), except: w1 fully in fp8-e3m4 and the first 2048 of
    w2's columns in fp8-e3m4 (scale 96, folded into x2h and the er
    copy-out respectively).
  - weights pre-packed host-side into partition-major [128, chunks, cols]
    layouts and streamed in 6-chunk groups (one DMA + a long matmul burst
    per group) so the PE stays busy and the DMA queue runs near the
    360 GB/s cap.
  - sum_weights GEMM1 column-sharded (each core owns 1536 columns of
    sw_w1).
  - ONE collective: AllGather of [er_e (64x3072) || z-partial (64x8)] in
    fp16. Cross-core wire DMAs are starved while the weight stream
    saturates the DMA engines, so pipelined AllReduces are useless here;
    a single post-stream AllGather whose input is only ready at stream
    end wastes nothing. Each core then sums z partials, softmaxes, and
    combines ws = sum_e wv[:,e]*er_e on DVE.
  - classification head sharded: GEMM1 column-shard (384 cols of ch_w1),
    GEMM2 contraction-shard (384 rows of ch_w2); per-core (64,1000)
    partials are summed on the host (+ ch_b2).
"""

import numpy as np
import ml_dtypes

import concourse.bass as bass
from concourse import bacc
import concourse.mybir as mybir
import concourse.tile as tile
from concourse.bass import ts, ds
from concourse.bass_utils import run_bass_kernel_spmd
from concourse.masks import make_identity

B, N, D, E, K, C = 64, 32, 384, 8, 8, 1000
KD, ND = K * D, N * D          # 3072, 12288
P = 128
NTOK = B * N                   # 2048
SWC = ND // E                  # 1536 sum-weights columns per core
CH1C = KD // E                 # 384 head-GEMM1 columns per core
KCE = KD // P                  # 24 k-chunks, expert GEMMs
KCS = ND // P                  # 96 k-chunks, sum-weights GEMM1
KCH = SWC // P                 # 12 k-chunks, z GEMM
NCORES = 8

G = 6                          # k-chunks per weight DMA group
W2F8C = 2048                   # fp8 columns of w2 (rest fp16)
S1 = 96.0                      # w1 fp8 scale (folded into x2h)
S2 = 96.0                      # w2 fp8 scale (folded into er copy-out)
AGW = KD + E                   # AllGather payload width (er || z)

F32 = mybir.dt.float32
F16 = mybir.dt.float16
BF16 = mybir.dt.bfloat16
F8E3 = mybir.dt.float8e3
U32 = mybir.dt.uint32
GELU = mybir.ActivationFunctionType.Gelu
EXP = mybir.ActivationFunctionType.Exp
X_AX = mybir.AxisListType.X
ADD = mybir.AluOpType.add
BYPASS = mybir.AluOpType.bypass
f16 = np.float16
bf16 = ml_dtypes.bfloat16
e3m4 = ml_dtypes.float8_e3m4


def _build(include_bias: bool) -> bass.Bass:
    nc = bacc.Bacc("TRN2", num_devices=NCORES)

    # xt: x.T in fp32 (router must match the reference's fp32 top-8 exactly;
    # a bf16 hi/lo split was ~8e-6 off and flipped one near-tie), emb_e
    # appended as column 2048
    xt = nc.dram_tensor("xt", [P, 3, NTOK + 1], F32, kind="ExternalInput")
    x2h = nc.dram_tensor("x2h", [NTOK, D], F16, kind="ExternalInput")
    xft = nc.dram_tensor("xft", [P, KCS, B], F16, kind="ExternalInput")
    sw1p = nc.dram_tensor("sw1p", [P, KCS, SWC], F16, kind="ExternalInput")
    w1p = nc.dram_tensor("w1p", [P, KCE, KD], F8E3, kind="ExternalInput")
    w2p8 = nc.dram_tensor("w2p8", [P, KCE, W2F8C], F8E3, kind="ExternalInput")
    w2p16 = nc.dram_tensor(
        "w2p16", [P, KCE, KD - W2F8C], F16, kind="ExternalInput")
    sw2p = nc.dram_tensor("sw2p", [P, KCH, E], F16, kind="ExternalInput")
    ch1p = nc.dram_tensor("ch1p", [P, KCE, CH1C], F16, kind="ExternalInput")
    ch2p = nc.dram_tensor("ch2p", [P, 3, C], F16, kind="ExternalInput")
    if include_bias:
        b1d = nc.dram_tensor("b1d", [1, KD], F32, kind="ExternalInput")
        b2d = nc.dram_tensor("b2d", [1, KD], F32, kind="ExternalInput")
        swb1d = nc.dram_tensor("swb1d", [1, SWC], F32, kind="ExternalInput")
        swb2d = nc.dram_tensor("swb2d", [1, E], F32, kind="ExternalInput")
        chb1d = nc.dram_tensor("chb1d", [1, CH1C], F32, kind="ExternalInput")
    outp = nc.dram_tensor("outp", [B, C], F32, kind="ExternalOutput")

    with tile.TileContext(nc) as tc:
        with (
            tc.tile_pool(name="consts", bufs=1) as consts,
            tc.tile_pool(name="acts", bufs=1) as acts,
            tc.tile_pool(name="wpool", bufs=4) as wpool,
            tc.tile_pool(name="ergp", bufs=2) as ergp,
            tc.tile_pool(name="ps_mm", bufs=6, space="PSUM") as ps_mm,
            tc.tile_pool(name="ps_tr", bufs=2, space="PSUM") as ps_tr,
            tc.tile_pool(name="dram", bufs=1, space="DRAM") as dram,
        ):
            # ---- constants / persistent activations ----
            ident = consts.tile([P, P], F16)
            make_identity(nc, ident[:])
            xft_sb = consts.tile([P, KCS, B], F16)
            nc.sync.dma_start(xft_sb[:], xft[:])
            sw2_sb = consts.tile([P, KCH, E], F16)
            nc.sync.dma_start(sw2_sb[:], sw2p[:])
            pwarm = ps_tr.tile([P, B], F16, name="pwarm", tag="pt")
            nc.tensor.transpose(pwarm[:32, :32], ident[:32, :32], ident[:32, :32])
            if include_bias:
                b1_sb = consts.tile([B, KD], F32)
                nc.scalar.dma_start(b1_sb[:], b1d[0:1, :].to_broadcast([B, KD]))
                b2_sb = consts.tile([B, KD], F32)
                nc.scalar.dma_start(b2_sb[:], b2d[0:1, :].to_broadcast([B, KD]))
                swb1_sb = consts.tile([B, SWC], F32)
                nc.scalar.dma_start(swb1_sb[:], swb1d[0:1, :].to_broadcast([B, SWC]))
                swb2_sb = consts.tile([B, E], F32)
                nc.scalar.dma_start(swb2_sb[:], swb2d[0:1, :].to_broadcast([B, E]))
                chb1_sb = consts.tile([B, CH1C], F32)
                nc.scalar.dma_start(chb1_sb[:], chb1d[0:1, :].to_broadcast([B, CH1C]))

            # ---- phase 1: sum-weights GEMM1, 16 groups of 6 k-chunks ----
            pms = [ps_mm.tile([B, 512], F32, name=f"pms{n}", tag="pm")
                   for n in range(3)]

            def sw_group(g):
                wt = wpool.tile([P, G, SWC], F16, name="wt", tag="wt")
                nc.sync.dma_start(wt[:], sw1p[:, ts(g, G), :])
                for cg in range(G):
                    c = g * G + cg
                    for n in range(3):
                        nc.tensor.matmul(
                            pms[n][:], xft_sb[:, c, :], wt[:, cg, ts(n, 512)],
                            start=(c == 0), stop=(c == KCS - 1),
                        )

            # xt rides the scalar queue so the sw weight stream never pauses
            xt_sb = acts.tile([P, 3, NTOK + 1], F32)
            nc.scalar.dma_start(xt_sb[:], xt[:])
            sw_group(0)
            sw_group(1)

            # ---- router: logits = x @ emb_e, fp32 (emb packed as col 2048) ----
            lg_flat = acts.tile([1, NTOK], F32)
            for nt in range(4):
                pr = ps_mm.tile([B, 512], F32, name="pr", tag="pm")
                for c in range(3):
                    nc.tensor.matmul(
                        pr[:1, :], xt_sb[:, c, NTOK : NTOK + 1],
                        xt_sb[:, c, ts(nt, 512)],
                        start=(c == 0), stop=(c == 2),
                    )
                nc.vector.tensor_copy(lg_flat[:, ts(nt, 512)], pr[:1, :])
            lg_dram = dram.tile([1, NTOK], F32)
            nc.scalar.dma_start(lg_dram[:], lg_flat[:])
            lg_bn = acts.tile([B, N], F32)
            nc.scalar.dma_start(lg_bn[:], lg_dram[:].rearrange("x (b n) -> (x b) n", b=B))

            # ---- top-8 tokens per row + gather (overlaps sw streaming) ----
            vals8 = acts.tile([B, 8], F32)
            idx8 = acts.tile([B, 8], U32)
            nc.vector.max(out=vals8[:], in_=lg_bn[:])
            nc.vector.max_index(out=idx8[:], in_max=vals8[:], in_values=lg_bn[:])
            base = acts.tile([B, 1], U32)
            nc.gpsimd.iota(base[:], pattern=[[0, 1]], base=0, channel_multiplier=N)
            off = acts.tile([B, 8], U32)
            nc.vector.tensor_tensor(
                out=off[:], in0=idx8[:], in1=base[:].to_broadcast([B, 8]), op=ADD
            )
            sel = acts.tile([B, K, D], F16)
            for k in range(K):
                nc.gpsimd.indirect_dma_start(
                    out=sel[:, k, :], out_offset=None,
                    in_=x2h[:],
                    in_offset=bass.IndirectOffsetOnAxis(ap=off[:, k : k + 1], axis=0),
                )
            sel_flat = sel[:].rearrange("b k d -> b (k d)")

            # ---- rest of the sw stream; selT transposes sprinkled in ----
            selT = acts.tile([P, KCE, B], F16)
            for g in range(2, KCS // G):
                sw_group(g)
            # selT transposes at the phase boundary: sel is long gathered, so
            # the PE never stalls on the router->top8->gather chain mid-stream
            for c in range(KCE):
                pt = ps_tr.tile([P, B], F16, name="pt", tag="pt")
                nc.tensor.transpose(pt[:], sel_flat[:, ts(c, P)], ident[:B, :B])
                nc.vector.tensor_copy(selT[:, c, :], pt[:])
            ch2_sb = consts.tile([P, 3, C], F16)
            nc.sync.dma_start(ch2_sb[:], ch2p[:])

            # ---- h1 = gelu(sw psums); z partial [64, 8] ----
            h1 = acts.tile([B, SWC], F16)
            for n in range(3):
                if include_bias:
                    nc.vector.tensor_add(pms[n][:], pms[n][:], swb1_sb[:, ts(n, 512)])
                nc.scalar.activation(h1[:, ts(n, 512)], pms[n][:], GELU)
            h1T = acts.tile([P, KCH, B], F16)
            for c in range(KCH):
                pt = ps_tr.tile([P, B], F16, name="pt", tag="pt")
                nc.tensor.transpose(pt[:], h1[:, ts(c, P)], ident[:B, :B])
                nc.vector.tensor_copy(h1T[:, c, :], pt[:])
            pz = ps_mm.tile([B, E], F32, name="pz", tag="pm")
            for c in range(KCH):
                nc.tensor.matmul(
                    pz[:], h1T[:, c, :], sw2_sb[:, c, :],
                    start=(c == 0), stop=(c == KCH - 1),
                )
            # z partials go in their own tiny early AllReduce; its wire DMAs
            # are starved while the weight stream runs, but it drains during
            # the stream's dips/end and is done before the big er AllGather
            # lands, so the softmax is off the critical path.
            z_sb = acts.tile([B, E], F32)
            nc.vector.tensor_copy(z_sb[:], pz[:])
            zin = dram.tile([B, E], F32)
            zout = dram.tile([B, E], F32)
            nc.gpsimd.dma_start(zin[:], z_sb[:])
            nc.gpsimd.collective_compute(
                "AllReduce", ADD, replica_groups=[list(range(NCORES))],
                ins=[zin[:].opt()], outs=[zout[:].opt()],
            )

            # ---- phase 2: expert GEMM1, fp8 w1 stream (4 groups of 6) ----
            h = acts.tile([B, KD], F16)
            pme = [ps_mm.tile([B, 512], F32, name=f"pme{n}", tag="pm")
                   for n in range(6)]
            for g in range(KCE // G):
                wt8 = wpool.tile([P, G, KD], F8E3, name="wt8", tag="wt")
                nc.sync.dma_start(wt8[:], w1p[:, ts(g, G), :])
                for cg in range(G):
                    c = g * G + cg
                    for n in range(6):
                        nc.tensor.matmul(
                            pme[n][:], selT[:, c, :], wt8[:, cg, ts(n, 512)],
                            start=(c == 0), stop=(c == KCE - 1),
                        )
            last_gelu = None
            for n in range(6):
                if include_bias:
                    nc.vector.tensor_add(pme[n][:], pme[n][:], b1_sb[:, ts(n, 512)])
                last_gelu = nc.scalar.activation(h[:, ts(n, 512)], pme[n][:], GELU)
            hT = acts.tile([P, KCE, B], F16)
            for c in range(KCE):
                pt = ps_tr.tile([P, B], F16, name="pt", tag="pt")
                nc.tensor.transpose(pt[:], h[:, ts(c, P)], ident[:B, :B])
                nc.vector.tensor_copy(hT[:, c, :], pt[:])

            # ---- phase 3: expert GEMM2, single pass, 6 psum banks ----
            # cols 0:2048 fp8 (scale S2 undone at copy-out), 2048:3072 fp16
            ch1_sb = acts.tile([P, KCE, CH1C], F16)
            pm2 = [ps_mm.tile([B, 512], F32, name=f"pm2{n}", tag="pm")
                   for n in range(6)]
            last_ermm = None
            for g in range(KCE // G):
                wt2a = wpool.tile([P, G, W2F8C], F8E3, name="wt2a", tag="wt")
                nc.sync.dma_start(wt2a[:], w2p8[:, ts(g, G), :])
                wt2b = wpool.tile([P, G, KD - W2F8C], F16, name="wt2b", tag="wt")
                nc.sync.dma_start(wt2b[:], w2p16[:, ts(g, G), :])
                for cg in range(G):
                    c = g * G + cg
                    for n in range(6):
                        if n < 4:
                            rhs = wt2a[:, cg, ts(n, 512)]
                        else:
                            rhs = wt2b[:, cg, ts(n - 4, 512)]
                        last_ermm = nc.tensor.matmul(
                            pm2[n][:], hT[:, c, :], rhs,
                            start=(c == 0), stop=(c == KCE - 1),
                        )
                # stream the head GEMM1 weights behind the w2 groups
                if g < 3:
                    nc.sync.dma_start(ch1_sb[:, ts(g, 8), :], ch1p[:, ts(g, 8), :])
            # er copy-out (1/S2 undone on the fp8 columns)
            er_sb = acts.tile([B, KD], F16)
            last_erc = None
            for n in range(6):
                if include_bias:
                    stage = acts.tile([B, 512], F32, name=f"stage{n}")
                    sc = (1.0 / S2) if n < 4 else 1.0
                    nc.vector.tensor_scalar_mul(stage[:], pm2[n][:], sc)
                    nc.vector.tensor_add(stage[:], stage[:], b2_sb[:, ts(n, 512)])
                    last_erc = nc.vector.tensor_copy(er_sb[:, ts(n, 512)], stage[:])
                elif n < 4:
                    last_erc = nc.vector.tensor_scalar_mul(
                        er_sb[:, ts(n, 512)], pm2[n][:], 1.0 / S2)
                else:
                    last_erc = nc.vector.tensor_copy(er_sb[:, ts(n, 512)], pm2[n][:])

            # ---- the big collective: AllGather of er ----
            cin = dram.tile([B, KD], F16)
            gout = dram.tile([NCORES * B, KD], F16)
            cin_dma = nc.scalar.dma_start(cin[:], er_sb[:])
            nc.gpsimd.collective_compute(
                "AllGather", BYPASS, replica_groups=[list(range(NCORES))],
                ins=[cin[:].opt()], outs=[gout[:].opt()],
            )

            # ---- softmax over experts -> wv, broadcast to wvb ----
            zb = acts.tile([B, E], F32)
            zb_dma = nc.gpsimd.dma_start(zb[:], zout[:])
            tile.add_dep_helper(zb_dma.ins, cin_dma.ins, sync=False,
                                reason="zb read after AG send kickoff")
            if include_bias:
                nc.vector.tensor_add(zb[:], zb[:], swb2_sb[:])
            mx = acts.tile([B, 1], F32)
            mx_i = nc.vector.reduce_max(mx[:], zb[:], axis=X_AX)
            tile.add_dep_helper(mx_i.ins, last_erc.ins, sync=False,
                                reason="softmax after er transpose copies on DVE")
            nmx = acts.tile([B, 1], F32)
            nc.vector.tensor_scalar_mul(nmx[:], mx[:], -1.0)
            exps = acts.tile([B, E], F32)
            exp_i = nc.scalar.activation(exps[:], zb[:], EXP, bias=nmx[:])
            tile.add_dep_helper(exp_i.ins, last_gelu.ins, sync=False,
                                reason="Exp after expert gelus on ACT")
            sm = acts.tile([B, 1], F32)
            nc.vector.reduce_sum(sm[:], exps[:], axis=X_AX)
            rs = acts.tile([B, 1], F32)
            nc.vector.reciprocal(rs[:], sm[:])
            wv = acts.tile([B, E], F32)
            nc.vector.tensor_scalar_mul(wv[:], exps[:], rs[:])

            # ---- combine: ws = sum_e wv[:, e] * er_e (streamed, fp16) ----
            # [B, KD] layout so the per-expert weight is a per-partition
            # scalar (fast DVE tensor_scalar path).
            ws = acts.tile([B, KD], F16)
            t16 = acts.tile([B, KD], F16)
            rb_dmas = []
            for r in range(NCORES):
                erg = ergp.tile([B, KD], F16, name="erg", tag="erg")
                i = nc.scalar.dma_start(erg[:], gout[ds(r * B, B), :])
                rb_dmas.append(i)
                if r == 0:
                    nc.vector.tensor_scalar_mul(ws[:], erg[:], wv[:, 0:1])
                else:
                    nc.vector.tensor_scalar_mul(t16[:], erg[:], wv[:, r : r + 1])
                    nc.vector.tensor_add(ws[:], ws[:], t16[:])
            tile.add_dep_helper(rb_dmas[0].ins, cin_dma.ins, sync=False,
                                reason="readbacks after AG send on scalar q")

            # ---- head GEMM1 (column shard): hh = gelu(ws @ ch1_e) ----
            # transposes first (PE pipelines with the DVE copies), then the
            # matmul burst back-to-back — avoids 24 PE<->DVE sem round trips.
            wsT = acts.tile([P, KCE, B], F16)
            for c in range(KCE):
                pt = ps_tr.tile([P, B], F16, name="pt", tag="pt")
                tr_i = nc.tensor.transpose(pt[:], ws[:, ts(c, P)], ident[:B, :B])
                if c == 0:
                    tile.add_dep_helper(tr_i.ins, last_ermm.ins, sync=False,
                                        reason="head transposes after er matmuls on PE")
                nc.vector.tensor_copy(wsT[:, c, :], pt[:])
            pmh = ps_mm.tile([B, CH1C], F32, name="pmh", tag="pm")
            for c in range(KCE):
                nc.tensor.matmul(
                    pmh[:], wsT[:, c, :], ch1_sb[:, c, :],
                    start=(c == 0), stop=(c == KCE - 1),
                )
            if include_bias:
                nc.vector.tensor_add(pmh[:], pmh[:], chb1_sb[:])
            hh = acts.tile([B, CH1C], F16)
            nc.scalar.activation(hh[:], pmh[:], GELU)

            # ---- head GEMM2 (contraction shard): out_part = hh @ ch2_e ----
            hhT = acts.tile([P, 3, B], F16)
            for c in range(3):
                pt = ps_tr.tile([P, B], F16, name="pt", tag="pt")
                nc.tensor.transpose(pt[:], hh[:, ts(c, P)], ident[:B, :B])
                nc.vector.tensor_copy(hhT[:, c, :], pt[:])
            outsb = acts.tile([B, C], F32)
            for nn in range(2):
                pmo = ps_mm.tile([B, 512], F32, name="pmo", tag="pm")
                for c in range(3):
                    nc.tensor.matmul(
                        pmo[:, :500], hhT[:, c, :], ch2_sb[:, c, ds(nn * 500, 500)],
                        start=(c == 0), stop=(c == 2),
                    )
                nc.vector.tensor_copy(outsb[:, ds(nn * 500, 500)], pmo[:, :500])
            nc.sync.dma_start(outp[:], outsb[:])

    nc.finalize()
    return nc


_NC_CACHE: dict = {}


def _get_nc(include_bias: bool) -> bass.Bass:
    if include_bias not in _NC_CACHE:
        _NC_CACHE[include_bias] = _build(include_bias)
    return _NC_CACHE[include_bias]


def _pmajor(a, nchunk, dt):
    """[nchunk*128, cols] -> [128, nchunk, cols] partition-major, cast."""
    cols = a.shape[1]
    return np.ascontiguousarray(
        a.reshape(nchunk, P, cols).transpose(1, 0, 2)).astype(dt)


def _pack_inputs(inputs: dict, include_bias: bool) -> list[dict]:
    fp32 = np.float32
    x = np.ascontiguousarray(np.asarray(inputs["x"], dtype=fp32))  # (64,32,384)
    expert_emb = np.asarray(inputs["expert_emb"], dtype=fp32)      # (8,384)
    w1 = np.asarray(inputs["w1"], fp32)                            # (8,3072,3072)
    w2 = np.asarray(inputs["w2"], fp32)
    sw_w1 = np.asarray(inputs["sw_w1"], fp32)                      # (12288,12288)
    sw_w2 = np.asarray(inputs["sw_w2"], fp32)                      # (12288,8)
    ch_w1 = np.asarray(inputs["ch_w1"], fp32)                      # (3072,3072)
    ch_w2 = np.asarray(inputs["ch_w2"], fp32)                      # (3072,1000)

    x2 = x.reshape(NTOK, D)
    xt_base = x2.T.reshape(3, P, NTOK).transpose(1, 0, 2)          # (128,3,2048)
    x2h = (x2 / S1).astype(f16)                                    # (2048,384)
    xf = x.reshape(B, ND)
    xft_p = _pmajor(np.ascontiguousarray(xf.T), KCS, f16)          # (128,96,64)

    in_maps = []
    for e in range(NCORES):
        emb_p = expert_emb[e].reshape(3, P).T                      # (128,3)
        xt_p = np.ascontiguousarray(
            np.concatenate([xt_base, emb_p[:, :, None]], axis=2), dtype=fp32)
        sw1_e = np.ascontiguousarray(sw_w1[:, e * SWC:(e + 1) * SWC])
        sw1_p = _pmajor(sw1_e, KCS, f16)                           # (128,96,1536)
        w1_p = _pmajor(w1[e] * S1, KCE, e3m4)                      # (128,24,3072)
        w2p8 = _pmajor(w2[e][:, :W2F8C] * S2, KCE, e3m4)           # (128,24,2048)
        w2p16 = _pmajor(w2[e][:, W2F8C:], KCE, f16)                # (128,24,1024)
        sw2_e = np.ascontiguousarray(sw_w2[e * SWC:(e + 1) * SWC, :])
        sw2_p = _pmajor(sw2_e, KCH, f16)                           # (128,12,8)
        ch1_p = _pmajor(
            np.ascontiguousarray(ch_w1[:, e * CH1C:(e + 1) * CH1C]), KCE, f16)
        ch2_p = _pmajor(
            np.ascontiguousarray(ch_w2[e * CH1C:(e + 1) * CH1C, :]), 3, f16)
        m = {
            "xt": xt_p, "x2h": x2h, "xft": xft_p, "sw1p": sw1_p,
            "w1p": w1_p, "w2p8": w2p8, "w2p16": w2p16,
            "sw2p": sw2_p, "ch1p": ch1_p, "ch2p": ch2_p,
        }
        if include_bias:
            m["b1d"] = np.asarray(inputs["b1"][e], fp32).reshape(1, KD)
            m["b2d"] = np.asarray(inputs["b2"][e], fp32).reshape(1, KD)
            m["swb1d"] = np.asarray(
                inputs["sw_b1"], fp32).reshape(1, ND)[:, e * SWC:(e + 1) * SWC]
            m["swb2d"] = np.asarray(inputs["sw_b2"], fp32).reshape(1, E)
            m["chb1d"] = np.asarray(
                inputs["ch_b1"], fp32).reshape(1, KD)[:, e * CH1C:(e + 1) * CH1C]
        in_maps.append(m)
    return in_maps


def _need_bias(inputs) -> bool:
    return any(
        float(np.abs(np.asarray(inputs[k])).max()) != 0.0
        for k in ("b1", "b2", "sw_b1", "sw_b2", "ch_b1")
    )


def run(inputs: dict, **run_kwargs):
    """Run on the 8 cores; returns (full_output, BassKernelResults)."""
    include_bias = _need_bias(inputs)
    nc = _get_nc(include_bias)
    in_maps = _pack_inputs(inputs, include_bias)
    res = run_bass_kernel_spmd(nc, in_maps, core_ids=list(range(NCORES)), **run_kwargs)
    out = np.zeros((B, C), dtype=np.float64)
    for e in range(NCORES):
        out += res.results[e]["outp"].astype(np.float64)
    out += np.asarray(inputs["ch_b2"], np.float64)
    return out.astype(np.float32), res


def kernel(**inputs) -> np.ndarray:
    out, _ = run(inputs)
    return out


# revision 31
# speedup vs baseline: 1.0734x; 1.0396x over previous
"""Expert-choice MoE kernel for 8 Trainium2 NeuronCores (expert-parallel).

Decomposition (core e handles expert e):
  - router logits x . emb_e computed as bf16 hi/lo split (exact to ~2^-18)
    on PE; top-8 token indices per batch row via DVE max8/max_index; token
    gather via indirect DMA.
  - all activations/weights in fp16 (same bytes/speed as bf16, 8x less
    rounding noise), except: w1 fully in fp8-e3m4 and the first 2048 of
    w2's columns in fp8-e3m4 (scale 96, folded into x2h and the er
    copy-out respectively).
  - weights pre-packed host-side into partition-major [128, chunks, cols]
    layouts and streamed in 6-chunk groups (one DMA + a long matmul burst
    per group) so the PE stays busy and the DMA queue runs near the
    360 GB/s cap.
  - sum_weights GEMM1 column-sharded (each core owns 1536 columns of
    sw_w1).
  - ONE collective: AllGather of [er_e (64x3072) || z-partial (64x8)] in
    fp16. Cross-core wire DMAs are starved while the weight stream
    saturates the DMA engines, so pipelined AllReduces are useless here;
    a single post-stream AllGather whose input is only ready at stream
    end wastes nothing. Each core then sums z partials, softmaxes, and
    combines ws = sum_e wv[:,e]*er_e on DVE.
  - classification head sharded: GEMM1 column-shard (384 cols of ch_w1),
    GEMM2 contraction-shard (384 rows of ch_w2); per-core (64,1000)
    partials are summed on the host (+ ch_b2).
"""

import numpy as np
import ml_dtypes

import concourse.bass as bass
from concourse import bacc
import concourse.mybir as mybir
import concourse.tile as tile
from concourse.bass import ts, ds
from concourse.bass_utils import run_bass_kernel_spmd
from concourse.masks import make_identity

B, N, D, E, K, C = 64, 32, 384, 8, 8, 1000
KD, ND = K * D, N * D          # 3072, 12288
P = 128
NTOK = B * N                   # 2048
SWC = ND // E                  # 1536 sum-weights columns per core
CH1C = KD // E                 # 384 head-GEMM1 columns per core
KCE = KD // P                  # 24 k-chunks, expert GEMMs
KCS = ND // P                  # 96 k-chunks, sum-weights GEMM1
KCH = SWC // P                 # 12 k-chunks, z GEMM
NCORES = 8

G = 6                          # k-chunks per weight DMA group
W2F8C = 2048                   # fp8 columns of w2 (rest fp16)
S1 = 96.0                      # w1 fp8 scale (folded into x2h)
S2 = 96.0                      # w2 fp8 scale (folded into er copy-out)
AGW = KD + E                   # AllGather payload width (er || z)

F32 = mybir.dt.float32
F16 = mybir.dt.float16
BF16 = mybir.dt.bfloat16
F8E3 = mybir.dt.float8e3
U32 = mybir.dt.uint32
GELU = mybir.ActivationFunctionType.Gelu
EXP = mybir.ActivationFunctionType.Exp
X_AX = mybir.AxisListType.X
ADD = mybir.AluOpType.add
BYPASS = mybir.AluOpType.bypass
f16 = np.float16
bf16 = ml_dtypes.bfloat16
e3m4 = ml_dtypes.float8_e3m4


def _build(include_bias: bool) -> bass.Bass:
    nc = bacc.Bacc("TRN2", num_devices=NCORES)

    # xt: x.T in fp32 (router must match the reference's fp32 top-8 exactly;
    # a bf16 hi/lo split was ~8e-6 off and flipped one near-tie), emb_e
    # appended as column 2048
    xt = nc.dram_tensor("xt", [P, 3, NTOK + 1], F32, kind="ExternalInput")
    x2h = nc.dram_tensor("x2h", [NTOK, D], F16, kind="ExternalInput")
    xft = nc.dram_tensor("xft", [P, KCS, B], F16, kind="ExternalInput")
    sw1p = nc.dram_tensor("sw1p", [P, KCS, SWC], F16, kind="ExternalInput")
    w1p = nc.dram_tensor("w1p", [P, KCE, KD], F8E3, kind="ExternalInput")
    w2p8 = nc.dram_tensor("w2p8", [P, KCE, W2F8C], F8E3, kind="ExternalInput")
    w2p16 = nc.dram_tensor(
        "w2p16", [P, KCE, KD - W2F8C], F16, kind="ExternalInput")
    sw2p = nc.dram_tensor("sw2p", [P, KCH, E], F16, kind="ExternalInput")
    ch1p = nc.dram_tensor("ch1p", [P, KCE, CH1C], F16, kind="ExternalInput")
    ch2p = nc.dram_tensor("ch2p", [P, 3, C], F16, kind="ExternalInput")
    if include_bias:
        b1d = nc.dram_tensor("b1d", [1, KD], F32, kind="ExternalInput")
        b2d = nc.dram_tensor("b2d", [1, KD], F32, kind="ExternalInput")
        swb1d = nc.dram_tensor("swb1d", [1, SWC], F32, kind="ExternalInput")
        swb2d = nc.dram_tensor("swb2d", [1, E], F32, kind="ExternalInput")
        chb1d = nc.dram_tensor("chb1d", [1, CH1C], F32, kind="ExternalInput")
    outp = nc.dram_tensor("outp", [B, C], F32, kind="ExternalOutput")

    with tile.TileContext(nc) as tc:
        with (
            tc.tile_pool(name="consts", bufs=1) as consts,
            tc.tile_pool(name="acts", bufs=1) as acts,
            tc.tile_pool(name="wpool", bufs=4) as wpool,
            tc.tile_pool(name="ergp", bufs=4) as ergp,
            tc.tile_pool(name="ps_mm", bufs=6, space="PSUM") as ps_mm,
            tc.tile_pool(name="ps_tr", bufs=2, space="PSUM") as ps_tr,
            tc.tile_pool(name="dram", bufs=1, space="DRAM") as dram,
        ):
            # ---- constants / persistent activations ----
            ident = consts.tile([P, P], F16)
            make_identity(nc, ident[:])
            identb = consts.tile([P, B], F16)
            make_identity(nc, identb[B:P, :])
            xft_sb = consts.tile([P, KCS, B], F16)
            nc.sync.dma_start(xft_sb[:], xft[:])
            sw2_sb = consts.tile([P, KCH, E], F16)
            nc.sync.dma_start(sw2_sb[:], sw2p[:])
            pwarm = ps_tr.tile([P, B], F16, name="pwarm", tag="pt")
            nc.tensor.transpose(pwarm[:32, :32], ident[:32, :32], ident[:32, :32])
            if include_bias:
                b1_sb = consts.tile([B, KD], F32)
                nc.scalar.dma_start(b1_sb[:], b1d[0:1, :].to_broadcast([B, KD]))
                b2_sb = consts.tile([B, KD], F32)
                nc.scalar.dma_start(b2_sb[:], b2d[0:1, :].to_broadcast([B, KD]))
                swb1_sb = consts.tile([B, SWC], F32)
                nc.scalar.dma_start(swb1_sb[:], swb1d[0:1, :].to_broadcast([B, SWC]))
                swb2_sb = consts.tile([B, E], F32)
                nc.scalar.dma_start(swb2_sb[:], swb2d[0:1, :].to_broadcast([B, E]))
                chb1_sb = consts.tile([B, CH1C], F32)
                nc.scalar.dma_start(chb1_sb[:], chb1d[0:1, :].to_broadcast([B, CH1C]))

            # ---- phase 1: sum-weights GEMM1, 16 groups of 6 k-chunks ----
            pms = [ps_mm.tile([B, 512], F32, name=f"pms{n}", tag="pm")
                   for n in range(3)]

            def sw_group(g):
                wt = wpool.tile([P, G, SWC], F16, name="wt", tag="wt")
                nc.sync.dma_start(wt[:], sw1p[:, ts(g, G), :])
                for cg in range(G):
                    c = g * G + cg
                    for n in range(3):
                        nc.tensor.matmul(
                            pms[n][:], xft_sb[:, c, :], wt[:, cg, ts(n, 512)],
                            start=(c == 0), stop=(c == KCS - 1),
                        )

            # xt rides the scalar queue so the sw weight stream never pauses
            xt_sb = acts.tile([P, 3, NTOK + 1], F32)
            nc.scalar.dma_start(xt_sb[:], xt[:])
            sw_group(0)
            sw_group(1)

            # ---- router: logits = x @ emb_e, fp32 (emb packed as col 2048).
            # Emitted in quarters between sw groups: one contiguous 20us blob
            # of 4-cycle/row fp32 matmuls stalls the PE behind the stream.
            lg_flat = acts.tile([1, NTOK], F32)

            def router_quarter(nt):
                pr = ps_mm.tile([B, 512], F32, name="pr", tag="pm")
                for c in range(3):
                    nc.tensor.matmul(
                        pr[:1, :], xt_sb[:, c, NTOK : NTOK + 1],
                        xt_sb[:, c, ts(nt, 512)],
                        start=(c == 0), stop=(c == 2),
                    )
                nc.vector.tensor_copy(lg_flat[:, ts(nt, 512)], pr[:1, :])

            # ---- rest of the sw stream ----
            selT = acts.tile([P, KCE, B], F16)
            for g in range(2, KCS // G):
                sw_group(g)
                if 2 <= g <= 5:
                    router_quarter(g - 2)
            # ---- top-8 tokens per row + gather (overlaps sw streaming) ----
            lg_dram = dram.tile([1, NTOK], F32)
            nc.scalar.dma_start(lg_dram[:], lg_flat[:])
            lg_bn = acts.tile([B, N], F32)
            nc.scalar.dma_start(lg_bn[:], lg_dram[:].rearrange("x (b n) -> (x b) n", b=B))
            vals8 = acts.tile([B, 8], F32)
            idx8 = acts.tile([B, 8], U32)
            nc.vector.max(out=vals8[:], in_=lg_bn[:])
            nc.vector.max_index(out=idx8[:], in_max=vals8[:], in_values=lg_bn[:])
            base = acts.tile([B, 1], U32)
            nc.gpsimd.iota(base[:], pattern=[[0, 1]], base=0, channel_multiplier=N)
            off = acts.tile([B, 8], U32)
            nc.vector.tensor_tensor(
                out=off[:], in0=idx8[:], in1=base[:].to_broadcast([B, 8]), op=ADD
            )
            sel = acts.tile([B, K, D], F16)
            for k in range(K):
                nc.gpsimd.indirect_dma_start(
                    out=sel[:, k, :], out_offset=None,
                    in_=x2h[:],
                    in_offset=bass.IndirectOffsetOnAxis(ap=off[:, k : k + 1], axis=0),
                )
            sel_flat = sel[:].rearrange("b k d -> b (k d)")

            # selT transposes at the phase boundary: sel is long gathered, so
            # the PE never stalls on the router->top8->gather chain mid-stream
            for c in range(KCE):
                pt = ps_tr.tile([P, B], F16, name="pt", tag="pt")
                nc.tensor.transpose(pt[:], sel_flat[:, ts(c, P)], ident[:B, :B])
                nc.vector.tensor_copy(selT[:, c, :], pt[:])
            ch2_sb = consts.tile([P, 3, C], F16)
            nc.sync.dma_start(ch2_sb[:], ch2p[:])

            # ---- h1 = gelu(sw psums); z partial [64, 8] ----
            h1 = acts.tile([B, SWC], F16)
            for n in range(3):
                if include_bias:
                    nc.vector.tensor_add(pms[n][:], pms[n][:], swb1_sb[:, ts(n, 512)])
                nc.scalar.activation(h1[:, ts(n, 512)], pms[n][:], GELU)
            h1T = acts.tile([P, KCH, B], F16)
            for c in range(KCH):
                pt = ps_tr.tile([P, B], F16, name="pt", tag="pt")
                nc.tensor.transpose(pt[:], h1[:, ts(c, P)], ident[:B, :B])
                nc.vector.tensor_copy(h1T[:, c, :], pt[:])
            pz = ps_mm.tile([B, E], F32, name="pz", tag="pm")
            for c in range(KCH):
                nc.tensor.matmul(
                    pz[:], h1T[:, c, :], sw2_sb[:, c, :],
                    start=(c == 0), stop=(c == KCH - 1),
                )
            # z partials go in their own tiny early AllReduce; its wire DMAs
            # are starved while the weight stream runs, but it drains during
            # the stream's dips/end and is done before the big er AllGather
            # lands, so the softmax is off the critical path.
            z_sb = acts.tile([B, E], F32)
            nc.vector.tensor_copy(z_sb[:], pz[:])
            zin = dram.tile([B, E], F32)
            zout = dram.tile([B, E], F32)
            nc.gpsimd.dma_start(zin[:], z_sb[:])
            nc.gpsimd.collective_compute(
                "AllReduce", ADD, replica_groups=[list(range(NCORES))],
                ins=[zin[:].opt()], outs=[zout[:].opt()],
            )

            # ---- phase 2: expert GEMM1, fp8 w1 stream (4 groups of 6) ----
            h = acts.tile([B, KD], F16)
            pme = [ps_mm.tile([B, 512], F32, name=f"pme{n}", tag="pm")
                   for n in range(6)]
            for g in range(KCE // G):
                wt8 = wpool.tile([P, G, KD], F8E3, name="wt8", tag="wt")
                nc.sync.dma_start(wt8[:], w1p[:, ts(g, G), :])
                for cg in range(G):
                    c = g * G + cg
                    for n in range(6):
                        nc.tensor.matmul(
                            pme[n][:], selT[:, c, :], wt8[:, cg, ts(n, 512)],
                            start=(c == 0), stop=(c == KCE - 1),
                        )
            last_gelu = None
            for n in range(6):
                if include_bias:
                    nc.vector.tensor_add(pme[n][:], pme[n][:], b1_sb[:, ts(n, 512)])
                last_gelu = nc.scalar.activation(h[:, ts(n, 512)], pme[n][:], GELU)
            hT = acts.tile([P, KCE, B], F16)
            for c in range(KCE):
                pt = ps_tr.tile([P, B], F16, name="pt", tag="pt")
                nc.tensor.transpose(pt[:], h[:, ts(c, P)], ident[:B, :B])
                nc.vector.tensor_copy(hT[:, c, :], pt[:])

            # ---- phase 3: expert GEMM2, single pass, 6 psum banks ----
            # cols 0:2048 fp8 (scale S2 undone at copy-out), 2048:3072 fp16
            ch1_sb = acts.tile([P, KCE, CH1C], F16)
            pm2 = [ps_mm.tile([B, 512], F32, name=f"pm2{n}", tag="pm")
                   for n in range(6)]
            last_ermm = None
            for g in range(KCE // G):
                wt2a = wpool.tile([P, G, W2F8C], F8E3, name="wt2a", tag="wt")
                nc.sync.dma_start(wt2a[:], w2p8[:, ts(g, G), :])
                wt2b = wpool.tile([P, G, KD - W2F8C], F16, name="wt2b", tag="wt")
                nc.sync.dma_start(wt2b[:], w2p16[:, ts(g, G), :])
                for cg in range(G):
                    c = g * G + cg
                    for n in range(6):
                        if n < 4:
                            rhs = wt2a[:, cg, ts(n, 512)]
                        else:
                            rhs = wt2b[:, cg, ts(n - 4, 512)]
                        last_ermm = nc.tensor.matmul(
                            pm2[n][:], hT[:, c, :], rhs,
                            start=(c == 0), stop=(c == KCE - 1),
                        )
                # stream the head GEMM1 weights behind the w2 groups
                if g < 3:
                    nc.sync.dma_start(ch1_sb[:, ts(g, 8), :], ch1p[:, ts(g, 8), :])
            # er copy-out (1/S2 undone on the fp8 columns)
            er_sb = acts.tile([B, KD], F16)
            last_erc = None
            for n in range(6):
                if include_bias:
                    stage = acts.tile([B, 512], F32, name=f"stage{n}")
                    sc = (1.0 / S2) if n < 4 else 1.0
                    nc.vector.tensor_scalar_mul(stage[:], pm2[n][:], sc)
                    nc.vector.tensor_add(stage[:], stage[:], b2_sb[:, ts(n, 512)])
                    last_erc = nc.vector.tensor_copy(er_sb[:, ts(n, 512)], stage[:])
                elif n < 4:
                    last_erc = nc.vector.tensor_scalar_mul(
                        er_sb[:, ts(n, 512)], pm2[n][:], 1.0 / S2)
                else:
                    last_erc = nc.vector.tensor_copy(er_sb[:, ts(n, 512)], pm2[n][:])

            # ---- the big collective: AllGather of er ----
            cin = dram.tile([B, KD], F16)
            gout = dram.tile([NCORES * B, KD], F16)
            cin_dma = nc.scalar.dma_start(cin[:], er_sb[:])
            nc.gpsimd.collective_compute(
                "AllGather", BYPASS, replica_groups=[list(range(NCORES))],
                ins=[cin[:].opt()], outs=[gout[:].opt()],
            )

            # ---- softmax over experts -> wv, broadcast to wvb ----
            zb = acts.tile([B, E], F32)
            zb_dma = nc.gpsimd.dma_start(zb[:], zout[:])
            tile.add_dep_helper(zb_dma.ins, cin_dma.ins, sync=False,
                                reason="zb read after AG send kickoff")
            if include_bias:
                nc.vector.tensor_add(zb[:], zb[:], swb2_sb[:])
            mx = acts.tile([B, 1], F32)
            mx_i = nc.vector.reduce_max(mx[:], zb[:], axis=X_AX)
            tile.add_dep_helper(mx_i.ins, last_erc.ins, sync=False,
                                reason="softmax after er transpose copies on DVE")
            nmx = acts.tile([B, 1], F32)
            nc.vector.tensor_scalar_mul(nmx[:], mx[:], -1.0)
            exps = acts.tile([B, E], F32)
            exp_i = nc.scalar.activation(exps[:], zb[:], EXP, bias=nmx[:])
            tile.add_dep_helper(exp_i.ins, last_gelu.ins, sync=False,
                                reason="Exp after expert gelus on ACT")
            sm = acts.tile([B, 1], F32)
            nc.vector.reduce_sum(sm[:], exps[:], axis=X_AX)
            rs = acts.tile([B, 1], F32)
            nc.vector.reciprocal(rs[:], sm[:])
            wv = acts.tile([B, E], F32)
            nc.vector.tensor_scalar_mul(wv[:], exps[:], rs[:])
            # duplicate wv onto both partition halves: wv2[h*64+b] = wv[b]
            wvd = dram.tile([B, E], F32)
            nc.gpsimd.dma_start(wvd[:], wv[:])
            wv2 = acts.tile([P, E], F32)
            nc.gpsimd.dma_start(wv2[0:B, :], wvd[:])
            nc.gpsimd.dma_start(wv2[B:P, :], wvd[:])

            # ---- combine: ws = sum_e wv[:, e] * er_e (fp16) ----
            # er read back as [128, 1536]: partitions 0:64 hold columns
            # 0:1536, partitions 64:128 hold columns 1536:3072 -- all 128 DVE
            # lanes work. Scaled copies alternate ACT/DVE; adds on DVE.
            HKD = KD // 2
            ws2 = acts.tile([P, HKD], F16)
            tmps = [acts.tile([P, HKD], F16, name=f"cmb{i}") for i in range(4)]
            rb_dmas = []
            for r in range(NCORES):
                erg = ergp.tile([P, HKD], F16, name="erg", tag="erg")
                i = nc.scalar.dma_start(erg[0:B, :], gout[ds(r * B, B), 0:HKD])
                nc.scalar.dma_start(erg[B:P, :], gout[ds(r * B, B), HKD:KD])
                rb_dmas.append(i)
                dst = ws2 if r == 0 else tmps[r % 4]
                if r % 2 == 0:
                    nc.vector.tensor_scalar_mul(dst[:], erg[:], wv2[:, r : r + 1])
                else:
                    nc.scalar.activation(dst[:], erg[:],
                                         mybir.ActivationFunctionType.Copy,
                                         scale=wv2[:, r : r + 1])
                if r > 0:
                    nc.vector.tensor_add(ws2[:], ws2[:], dst[:])
            tile.add_dep_helper(rb_dmas[0].ins, cin_dma.ins, sync=False,
                                reason="readbacks after AG send on scalar q")

            # ---- head GEMM1 (column shard): hh = gelu(ws @ ch1_e) ----
            # transposes first (PE pipelines with the DVE copies), then the
            # matmul burst back-to-back — avoids 24 PE<->DVE sem round trips.
            wsT = acts.tile([P, KCE, B], F16)
            for c in range(KCE):
                pt = ps_tr.tile([P, B], F16, name="pt", tag="pt")
                if c < KCE // 2:
                    tr_i = nc.tensor.transpose(
                        pt[:], ws2[0:B, ts(c, P)], ident[:B, :B])
                else:
                    tr_i = nc.tensor.transpose(
                        pt[:], ws2[B:P, ts(c - KCE // 2, P)], identb[B:P, :])
                if c == 0:
                    tile.add_dep_helper(tr_i.ins, last_ermm.ins, sync=False,
                                        reason="head transposes after er matmuls on PE")
                nc.vector.tensor_copy(wsT[:, c, :], pt[:])
            pmh = ps_mm.tile([B, CH1C], F32, name="pmh", tag="pm")
            for c in range(KCE):
                nc.tensor.matmul(
                    pmh[:], wsT[:, c, :], ch1_sb[:, c, :],
                    start=(c == 0), stop=(c == KCE - 1),
                )
            if include_bias:
                nc.vector.tensor_add(pmh[:], pmh[:], chb1_sb[:])
            hh = acts.tile([B, CH1C], F16)
            nc.scalar.activation(hh[:], pmh[:], GELU)

            # ---- head GEMM2 (contraction shard): out_part = hh @ ch2_e ----
            hhT = acts.tile([P, 3, B], F16)
            for c in range(3):
                pt = ps_tr.tile([P, B], F16, name="pt", tag="pt")
                nc.tensor.transpose(pt[:], hh[:, ts(c, P)], ident[:B, :B])
                nc.vector.tensor_copy(hhT[:, c, :], pt[:])
            outsb = acts.tile([B, C], F32)
            for nn in range(2):
                pmo = ps_mm.tile([B, 512], F32, name="pmo", tag="pm")
                for c in range(3):
                    nc.tensor.matmul(
                        pmo[:, :500], hhT[:, c, :], ch2_sb[:, c, ds(nn * 500, 500)],
                        start=(c == 0), stop=(c == 2),
                    )
                nc.vector.tensor_copy(outsb[:, ds(nn * 500, 500)], pmo[:, :500])
            nc.sync.dma_start(outp[:], outsb[:])

    nc.finalize()
    return nc


_NC_CACHE: dict = {}


def _get_nc(include_bias: bool) -> bass.Bass:
    if include_bias not in _NC_CACHE:
        _NC_CACHE[include_bias] = _build(include_bias)
    return _NC_CACHE[include_bias]


def _pmajor(a, nchunk, dt):
    """[nchunk*128, cols] -> [128, nchunk, cols] partition-major, cast."""
    cols = a.shape[1]
    return np.ascontiguousarray(
        a.reshape(nchunk, P, cols).transpose(1, 0, 2)).astype(dt)


def _pack_inputs(inputs: dict, include_bias: bool) -> list[dict]:
    fp32 = np.float32
    x = np.ascontiguousarray(np.asarray(inputs["x"], dtype=fp32))  # (64,32,384)
    expert_emb = np.asarray(inputs["expert_emb"], dtype=fp32)      # (8,384)
    w1 = np.asarray(inputs["w1"], fp32)                            # (8,3072,3072)
    w2 = np.asarray(inputs["w2"], fp32)
    sw_w1 = np.asarray(inputs["sw_w1"], fp32)                      # (12288,12288)
    sw_w2 = np.asarray(inputs["sw_w2"], fp32)                      # (12288,8)
    ch_w1 = np.asarray(inputs["ch_w1"], fp32)                      # (3072,3072)
    ch_w2 = np.asarray(inputs["ch_w2"], fp32)                      # (3072,1000)

    x2 = x.reshape(NTOK, D)
    xt_base = x2.T.reshape(3, P, NTOK).transpose(1, 0, 2)          # (128,3,2048)
    x2h = (x2 / S1).astype(f16)                                    # (2048,384)
    xf = x.reshape(B, ND)
    xft_p = _pmajor(np.ascontiguousarray(xf.T), KCS, f16)          # (128,96,64)

    in_maps = []
    for e in range(NCORES):
        emb_p = expert_emb[e].reshape(3, P).T                      # (128,3)
        xt_p = np.ascontiguousarray(
            np.concatenate([xt_base, emb_p[:, :, None]], axis=2), dtype=fp32)
        sw1_e = np.ascontiguousarray(sw_w1[:, e * SWC:(e + 1) * SWC])
        sw1_p = _pmajor(sw1_e, KCS, f16)                           # (128,96,1536)
        w1_p = _pmajor(w1[e] * S1, KCE, e3m4)                      # (128,24,3072)
        w2p8 = _pmajor(w2[e][:, :W2F8C] * S2, KCE, e3m4)           # (128,24,2048)
        w2p16 = _pmajor(w2[e][:, W2F8C:], KCE, f16)                # (128,24,1024)
        sw2_e = np.ascontiguousarray(sw_w2[e * SWC:(e + 1) * SWC, :])
        sw2_p = _pmajor(sw2_e, KCH, f16)                           # (128,12,8)
        ch1_p = _pmajor(
            np.ascontiguousarray(ch_w1[:, e * CH1C:(e + 1) * CH1C]), KCE, f16)
        ch2_p = _pmajor(
            np.ascontiguousarray(ch_w2[e * CH1C:(e + 1) * CH1C, :]), 3, f16)
        m = {
            "xt": xt_p, "x2h": x2h, "xft": xft_p, "sw1p": sw1_p,
            "w1p": w1_p, "w2p8": w2p8, "w2p16": w2p16,
            "sw2p": sw2_p, "ch1p": ch1_p, "ch2p": ch2_p,
        }
        if include_bias:
            m["b1d"] = np.asarray(inputs["b1"][e], fp32).reshape(1, KD)
            m["b2d"] = np.asarray(inputs["b2"][e], fp32).reshape(1, KD)
            m["swb1d"] = np.asarray(
                inputs["sw_b1"], fp32).reshape(1, ND)[:, e * SWC:(e + 1) * SWC]
            m["swb2d"] = np.asarray(inputs["sw_b2"], fp32).reshape(1, E)
            m["chb1d"] = np.asarray(
                inputs["ch_b1"], fp32).reshape(1, KD)[:, e * CH1C:(e + 1) * CH1C]
        in_maps.append(m)
    return in_maps


def _need_bias(inputs) -> bool:
    return any(
        float(np.abs(np.asarray(inputs[k])).max()) != 0.0
        for k in ("b1", "b2", "sw_b1", "sw_b2", "ch_b1")
    )


def run(inputs: dict, **run_kwargs):
    """Run on the 8 cores; returns (full_output, BassKernelResults)."""
    include_bias = _need_bias(inputs)
    nc = _get_nc(include_bias)
    in_maps = _pack_inputs(inputs, include_bias)
    res = run_bass_kernel_spmd(nc, in_maps, core_ids=list(range(NCORES)), **run_kwargs)
    out = np.zeros((B, C), dtype=np.float64)
    for e in range(NCORES):
        out += res.results[e]["outp"].astype(np.float64)
    out += np.asarray(inputs["ch_b2"], np.float64)
    return out.astype(np.float32), res


def kernel(**inputs) -> np.ndarray:
    out, _ = run(inputs)
    return out


# revision 34
# speedup vs baseline: 1.0968x; 1.0218x over previous
"""Expert-choice MoE kernel for 8 Trainium2 NeuronCores (expert-parallel).

Decomposition (core e handles expert e):
  - router logits x . emb_e computed as bf16 hi/lo split (exact to ~2^-18)
    on PE; top-8 token indices per batch row via DVE max8/max_index; token
    gather via indirect DMA.
  - all activations/weights in fp16 (same bytes/speed as bf16, 8x less
    rounding noise), except: w1 fully in fp8-e3m4 and the first 2048 of
    w2's columns in fp8-e3m4 (scale 96, folded into x2h and the er
    copy-out respectively).
  - weights pre-packed host-side into partition-major [128, chunks, cols]
    layouts and streamed in 6-chunk groups (one DMA + a long matmul burst
    per group) so the PE stays busy and the DMA queue runs near the
    360 GB/s cap.
  - sum_weights GEMM1 column-sharded (each core owns 1536 columns of
    sw_w1).
  - ONE collective: AllGather of [er_e (64x3072) || z-partial (64x8)] in
    fp16. Cross-core wire DMAs are starved while the weight stream
    saturates the DMA engines, so pipelined AllReduces are useless here;
    a single post-stream AllGather whose input is only ready at stream
    end wastes nothing. Each core then sums z partials, softmaxes, and
    combines ws = sum_e wv[:,e]*er_e on DVE.
  - classification head sharded: GEMM1 column-shard (384 cols of ch_w1),
    GEMM2 contraction-shard (384 rows of ch_w2); per-core (64,1000)
    partials are summed on the host (+ ch_b2).
"""

import numpy as np
import ml_dtypes

import concourse.bass as bass
from concourse import bacc
import concourse.mybir as mybir
import concourse.tile as tile
from concourse.bass import ts, ds
from concourse.bass_utils import run_bass_kernel_spmd
from concourse.masks import make_identity

B, N, D, E, K, C = 64, 32, 384, 8, 8, 1000
KD, ND = K * D, N * D          # 3072, 12288
P = 128
NTOK = B * N                   # 2048
SWC = ND // E                  # 1536 sum-weights columns per core
CH1C = KD // E                 # 384 head-GEMM1 columns per core
KCE = KD // P                  # 24 k-chunks, expert GEMMs
KCS = ND // P                  # 96 k-chunks, sum-weights GEMM1
KCH = SWC // P                 # 12 k-chunks, z GEMM
NCORES = 8

G = 6                          # k-chunks per weight DMA group
W2F8C = 2048                   # fp8 columns of w2 (rest fp16)
S1 = 96.0                      # w1 fp8 scale (folded into x2h)
S2 = 96.0                      # w2 fp8 scale (folded into er copy-out)
AGW = KD + E                   # AllGather payload width (er || z)

F32 = mybir.dt.float32
F16 = mybir.dt.float16
BF16 = mybir.dt.bfloat16
F8E3 = mybir.dt.float8e3
U32 = mybir.dt.uint32
GELU = mybir.ActivationFunctionType.Gelu
EXP = mybir.ActivationFunctionType.Exp
X_AX = mybir.AxisListType.X
ADD = mybir.AluOpType.add
BYPASS = mybir.AluOpType.bypass
f16 = np.float16
bf16 = ml_dtypes.bfloat16
e3m4 = ml_dtypes.float8_e3m4


def _build(include_bias: bool) -> bass.Bass:
    nc = bacc.Bacc("TRN2", num_devices=NCORES)

    # xt: x.T in fp32 (router must match the reference's fp32 top-8 exactly;
    # a bf16 hi/lo split was ~8e-6 off and flipped one near-tie), emb_e
    # appended as column 2048
    xt = nc.dram_tensor("xt", [P, 3, NTOK + 1], F32, kind="ExternalInput")
    x2h = nc.dram_tensor("x2h", [NTOK, D], F16, kind="ExternalInput")
    xft = nc.dram_tensor("xft", [P, KCS, B], F16, kind="ExternalInput")
    sw1p = nc.dram_tensor("sw1p", [P, KCS, SWC], F16, kind="ExternalInput")
    w1p = nc.dram_tensor("w1p", [P, KCE, KD], F8E3, kind="ExternalInput")
    w2p8 = nc.dram_tensor("w2p8", [P, KCE, W2F8C], F8E3, kind="ExternalInput")
    w2p16 = nc.dram_tensor(
        "w2p16", [P, KCE, KD - W2F8C], F16, kind="ExternalInput")
    sw2p = nc.dram_tensor("sw2p", [P, KCH, E], F16, kind="ExternalInput")
    ch1p = nc.dram_tensor("ch1p", [P, KCE, CH1C], F16, kind="ExternalInput")
    ch2p = nc.dram_tensor("ch2p", [P, 3, C], F16, kind="ExternalInput")
    if include_bias:
        b1d = nc.dram_tensor("b1d", [1, KD], F32, kind="ExternalInput")
        b2d = nc.dram_tensor("b2d", [1, KD], F32, kind="ExternalInput")
        swb1d = nc.dram_tensor("swb1d", [1, SWC], F32, kind="ExternalInput")
        swb2d = nc.dram_tensor("swb2d", [1, E], F32, kind="ExternalInput")
        chb1d = nc.dram_tensor("chb1d", [1, CH1C], F32, kind="ExternalInput")
    outp = nc.dram_tensor("outp", [B, C], F32, kind="ExternalOutput")

    with tile.TileContext(nc) as tc:
        with (
            tc.tile_pool(name="consts", bufs=1) as consts,
            tc.tile_pool(name="acts", bufs=1) as acts,
            tc.tile_pool(name="wpool", bufs=4) as wpool,
            tc.tile_pool(name="ergp", bufs=4) as ergp,
            tc.tile_pool(name="ps_mm", bufs=6, space="PSUM") as ps_mm,
            tc.tile_pool(name="ps_tr", bufs=2, space="PSUM") as ps_tr,
            tc.tile_pool(name="dram", bufs=1, space="DRAM") as dram,
        ):
            # ---- constants / persistent activations ----
            ident = consts.tile([P, P], F16)
            make_identity(nc, ident[:])
            identb = consts.tile([P, B], F16)
            make_identity(nc, identb[B:P, :])
            xft_sb = consts.tile([P, KCS, B], F16)
            nc.sync.dma_start(xft_sb[:], xft[:])
            sw2_sb = consts.tile([P, KCH, E], F16)
            nc.sync.dma_start(sw2_sb[:], sw2p[:])
            pwarm = ps_tr.tile([P, B], F16, name="pwarm", tag="pt")
            nc.tensor.transpose(pwarm[:32, :32], ident[:32, :32], ident[:32, :32])
            if include_bias:
                b1_sb = consts.tile([B, KD], F32)
                nc.scalar.dma_start(b1_sb[:], b1d[0:1, :].to_broadcast([B, KD]))
                b2_sb = consts.tile([B, KD], F32)
                nc.scalar.dma_start(b2_sb[:], b2d[0:1, :].to_broadcast([B, KD]))
                swb1_sb = consts.tile([B, SWC], F32)
                nc.scalar.dma_start(swb1_sb[:], swb1d[0:1, :].to_broadcast([B, SWC]))
                swb2_sb = consts.tile([B, E], F32)
                nc.scalar.dma_start(swb2_sb[:], swb2d[0:1, :].to_broadcast([B, E]))
                chb1_sb = consts.tile([B, CH1C], F32)
                nc.scalar.dma_start(chb1_sb[:], chb1d[0:1, :].to_broadcast([B, CH1C]))

            # ---- phase 1: sum-weights GEMM1, 16 groups of 6 k-chunks ----
            pms = [ps_mm.tile([B, 512], F32, name=f"pms{n}", tag="pm")
                   for n in range(3)]

            def sw_group(g):
                wt = wpool.tile([P, G, SWC], F16, name="wt", tag="wt")
                nc.sync.dma_start(wt[:], sw1p[:, ts(g, G), :])
                for cg in range(G):
                    c = g * G + cg
                    for n in range(3):
                        nc.tensor.matmul(
                            pms[n][:], xft_sb[:, c, :], wt[:, cg, ts(n, 512)],
                            start=(c == 0), stop=(c == KCS - 1),
                        )

            # xt rides the scalar queue so the sw weight stream never pauses
            xt_sb = acts.tile([P, 3, NTOK + 1], F32)
            nc.scalar.dma_start(xt_sb[:], xt[:])
            sw_group(0)
            sw_group(1)

            # ---- router: logits = x @ emb_e, fp32 (emb packed as col 2048).
            # Emitted in quarters between sw groups: one contiguous 20us blob
            # of 4-cycle/row fp32 matmuls stalls the PE behind the stream.
            lg_flat = acts.tile([1, NTOK], F32)

            def router_quarter(nt):
                pr = ps_mm.tile([B, 512], F32, name="pr", tag="pm")
                for c in range(3):
                    nc.tensor.matmul(
                        pr[:1, :], xt_sb[:, c, NTOK : NTOK + 1],
                        xt_sb[:, c, ts(nt, 512)],
                        start=(c == 0), stop=(c == 2),
                    )
                nc.vector.tensor_copy(lg_flat[:, ts(nt, 512)], pr[:1, :])

            # ---- rest of the sw stream ----
            selT = acts.tile([P, KCE, B], F16)
            for g in range(2, KCS // G):
                sw_group(g)
                if 2 <= g <= 5:
                    router_quarter(g - 2)
            # ---- top-8 tokens per row + gather (overlaps sw streaming) ----
            lg_dram = dram.tile([1, NTOK], F32)
            nc.scalar.dma_start(lg_dram[:], lg_flat[:])
            lg_bn = acts.tile([B, N], F32)
            nc.scalar.dma_start(lg_bn[:], lg_dram[:].rearrange("x (b n) -> (x b) n", b=B))
            vals8 = acts.tile([B, 8], F32)
            idx8 = acts.tile([B, 8], U32)
            nc.vector.max(out=vals8[:], in_=lg_bn[:])
            nc.vector.max_index(out=idx8[:], in_max=vals8[:], in_values=lg_bn[:])
            base = acts.tile([B, 1], U32)
            nc.gpsimd.iota(base[:], pattern=[[0, 1]], base=0, channel_multiplier=N)
            off = acts.tile([B, 8], U32)
            nc.vector.tensor_tensor(
                out=off[:], in0=idx8[:], in1=base[:].to_broadcast([B, 8]), op=ADD
            )
            sel = acts.tile([B, K, D], F16)
            for k in range(K):
                nc.gpsimd.indirect_dma_start(
                    out=sel[:, k, :], out_offset=None,
                    in_=x2h[:],
                    in_offset=bass.IndirectOffsetOnAxis(ap=off[:, k : k + 1], axis=0),
                )
            sel_flat = sel[:].rearrange("b k d -> b (k d)")

            # selT transposes at the phase boundary: sel is long gathered, so
            # the PE never stalls on the router->top8->gather chain mid-stream
            for c in range(KCE):
                pt = ps_tr.tile([P, B], F16, name="pt", tag="pt")
                nc.tensor.transpose(pt[:], sel_flat[:, ts(c, P)], ident[:B, :B])
                nc.vector.tensor_copy(selT[:, c, :], pt[:])
            ch2_sb = consts.tile([P, 3, C], F16)
            nc.sync.dma_start(ch2_sb[:], ch2p[:])

            # ---- h1 = gelu(sw psums); z partial [64, 8] ----
            h1 = acts.tile([B, SWC], F16)
            for n in range(3):
                if include_bias:
                    nc.vector.tensor_add(pms[n][:], pms[n][:], swb1_sb[:, ts(n, 512)])
                nc.scalar.activation(h1[:, ts(n, 512)], pms[n][:], GELU)
            h1T = acts.tile([P, KCH, B], F16)
            for c in range(KCH):
                pt = ps_tr.tile([P, B], F16, name="pt", tag="pt")
                nc.tensor.transpose(pt[:], h1[:, ts(c, P)], ident[:B, :B])
                nc.vector.tensor_copy(h1T[:, c, :], pt[:])
            pz = ps_mm.tile([B, E], F32, name="pz", tag="pm")
            for c in range(KCH):
                nc.tensor.matmul(
                    pz[:], h1T[:, c, :], sw2_sb[:, c, :],
                    start=(c == 0), stop=(c == KCH - 1),
                )
            # z partials go in their own tiny early AllReduce; its wire DMAs
            # are starved while the weight stream runs, but it drains during
            # the stream's dips/end and is done before the big er AllGather
            # lands, so the softmax is off the critical path.
            z_sb = acts.tile([B, E], F32)
            nc.vector.tensor_copy(z_sb[:], pz[:])
            zin = dram.tile([B, E], F32)
            zout = dram.tile([B, E], F32)
            nc.gpsimd.dma_start(zin[:], z_sb[:])
            nc.gpsimd.collective_compute(
                "AllReduce", ADD, replica_groups=[list(range(NCORES))],
                ins=[zin[:].opt()], outs=[zout[:].opt()],
            )

            # ---- phase 2: expert GEMM1, fp8 w1 stream (4 groups of 6) ----
            h = acts.tile([B, KD], F16)
            pme = [ps_mm.tile([B, 512], F32, name=f"pme{n}", tag="pm")
                   for n in range(6)]
            for g in range(KCE // G):
                wt8 = wpool.tile([P, G, KD], F8E3, name="wt8", tag="wt")
                nc.sync.dma_start(wt8[:], w1p[:, ts(g, G), :])
                for cg in range(G):
                    c = g * G + cg
                    for n in range(6):
                        nc.tensor.matmul(
                            pme[n][:], selT[:, c, :], wt8[:, cg, ts(n, 512)],
                            start=(c == 0), stop=(c == KCE - 1),
                        )
            last_gelu = None
            for n in range(6):
                if include_bias:
                    nc.vector.tensor_add(pme[n][:], pme[n][:], b1_sb[:, ts(n, 512)])
                last_gelu = nc.scalar.activation(h[:, ts(n, 512)], pme[n][:], GELU)
            hT = acts.tile([P, KCE, B], F16)
            for c in range(KCE):
                pt = ps_tr.tile([P, B], F16, name="pt", tag="pt")
                nc.tensor.transpose(pt[:], h[:, ts(c, P)], ident[:B, :B])
                nc.vector.tensor_copy(hT[:, c, :], pt[:])

            # ---- phase 3: expert GEMM2, single pass, 6 psum banks ----
            # cols 0:2048 fp8 (scale S2 undone at copy-out), 2048:3072 fp16
            ch1_sb = acts.tile([P, KCE, CH1C], F16)
            pm2 = [ps_mm.tile([B, 512], F32, name=f"pm2{n}", tag="pm")
                   for n in range(6)]
            last_ermm = None
            for g in range(KCE // G):
                wt2a = wpool.tile([P, G, W2F8C], F8E3, name="wt2a", tag="wt")
                nc.sync.dma_start(wt2a[:], w2p8[:, ts(g, G), :])
                wt2b = wpool.tile([P, G, KD - W2F8C], F16, name="wt2b", tag="wt")
                nc.sync.dma_start(wt2b[:], w2p16[:, ts(g, G), :])
                for cg in range(G):
                    c = g * G + cg
                    for n in range(6):
                        if n < 4:
                            rhs = wt2a[:, cg, ts(n, 512)]
                        else:
                            rhs = wt2b[:, cg, ts(n - 4, 512)]
                        last_ermm = nc.tensor.matmul(
                            pm2[n][:], hT[:, c, :], rhs,
                            start=(c == 0), stop=(c == KCE - 1),
                        )
                # stream the head GEMM1 weights behind the w2 groups
                if g < 3:
                    nc.sync.dma_start(ch1_sb[:, ts(g, 8), :], ch1p[:, ts(g, 8), :])
            # er copy-out (1/S2 undone on the fp8 columns)
            er_sb = acts.tile([B, KD], F16)
            last_erc = None
            for n in range(6):
                if include_bias:
                    stage = acts.tile([B, 512], F32, name=f"stage{n}")
                    sc = (1.0 / S2) if n < 4 else 1.0
                    nc.vector.tensor_scalar_mul(stage[:], pm2[n][:], sc)
                    nc.vector.tensor_add(stage[:], stage[:], b2_sb[:, ts(n, 512)])
                    last_erc = nc.vector.tensor_copy(er_sb[:, ts(n, 512)], stage[:])
                elif n < 4:
                    last_erc = nc.vector.tensor_scalar_mul(
                        er_sb[:, ts(n, 512)], pm2[n][:], 1.0 / S2)
                else:
                    last_erc = nc.vector.tensor_copy(er_sb[:, ts(n, 512)], pm2[n][:])

            # ---- the big collective: AllGather of er ----
            cin = dram.tile([B, KD], F16)
            gout = dram.tile([NCORES * B, KD], F16)
            cin_dma = nc.scalar.dma_start(cin[:], er_sb[:])
            nc.gpsimd.collective_compute(
                "AllGather", BYPASS, replica_groups=[list(range(NCORES))],
                ins=[cin[:].opt()], outs=[gout[:].opt()],
            )

            # ---- softmax over experts -> wv, broadcast to wvb ----
            zb = acts.tile([B, E], F32)
            zb_dma = nc.gpsimd.dma_start(zb[:], zout[:])
            tile.add_dep_helper(zb_dma.ins, cin_dma.ins, sync=False,
                                reason="zb read after AG send kickoff")
            if include_bias:
                nc.vector.tensor_add(zb[:], zb[:], swb2_sb[:])
            mx = acts.tile([B, 1], F32)
            mx_i = nc.vector.reduce_max(mx[:], zb[:], axis=X_AX)
            tile.add_dep_helper(mx_i.ins, last_erc.ins, sync=False,
                                reason="softmax after er transpose copies on DVE")
            nmx = acts.tile([B, 1], F32)
            nc.vector.tensor_scalar_mul(nmx[:], mx[:], -1.0)
            exps = acts.tile([B, E], F32)
            exp_i = nc.scalar.activation(exps[:], zb[:], EXP, bias=nmx[:])
            tile.add_dep_helper(exp_i.ins, last_gelu.ins, sync=False,
                                reason="Exp after expert gelus on ACT")
            gwarm = acts.tile([B, 1], F32)
            nc.scalar.activation(gwarm[:], mx[:], GELU)
            sm = acts.tile([B, 1], F32)
            nc.vector.reduce_sum(sm[:], exps[:], axis=X_AX)
            rs = acts.tile([B, 1], F32)
            nc.vector.reciprocal(rs[:], sm[:])
            wv = acts.tile([B, E], F32)
            nc.vector.tensor_scalar_mul(wv[:], exps[:], rs[:])
            # duplicate wv onto both partition halves: wv2[h*64+b] = wv[b]
            wvd = dram.tile([B, E], F32)
            nc.gpsimd.dma_start(wvd[:], wv[:])
            wv2 = acts.tile([P, E], F32)
            nc.gpsimd.dma_start(wv2[0:B, :], wvd[:])
            nc.gpsimd.dma_start(wv2[B:P, :], wvd[:])

            # ---- combine: ws = sum_e wv[:, e] * er_e (fp16) ----
            # er read back as [128, 1536]: partitions 0:64 hold columns
            # 0:1536, partitions 64:128 hold columns 1536:3072 -- all 128 DVE
            # lanes work. Scaled copies alternate ACT/DVE; adds on DVE.
            HKD = KD // 2
            ws2 = acts.tile([P, HKD], F16)
            tmps = [acts.tile([P, HKD], F16, name=f"cmb{i}") for i in range(4)]
            rb_dmas = []
            for r in range(NCORES):
                erg = ergp.tile([P, HKD], F16, name="erg", tag="erg")
                i = nc.scalar.dma_start(erg[0:B, :], gout[ds(r * B, B), 0:HKD])
                nc.scalar.dma_start(erg[B:P, :], gout[ds(r * B, B), HKD:KD])
                rb_dmas.append(i)
                dst = ws2 if r == 0 else tmps[r % 4]
                if r % 2 == 0:
                    nc.vector.tensor_scalar_mul(dst[:], erg[:], wv2[:, r : r + 1])
                else:
                    nc.scalar.activation(dst[:], erg[:],
                                         mybir.ActivationFunctionType.Copy,
                                         scale=wv2[:, r : r + 1])
                if r > 0:
                    nc.vector.tensor_add(ws2[:], ws2[:], dst[:])
            tile.add_dep_helper(rb_dmas[0].ins, cin_dma.ins, sync=False,
                                reason="readbacks after AG send on scalar q")

            # ---- head GEMM1 (column shard): hh = gelu(ws @ ch1_e) ----
            # transposes first (PE pipelines with the DVE copies), then the
            # matmul burst back-to-back — avoids 24 PE<->DVE sem round trips.
            wsT = acts.tile([P, KCE, B], F16)
            for c in range(KCE):
                pt = ps_tr.tile([P, B], F16, name="pt", tag="pt")
                if c < KCE // 2:
                    tr_i = nc.tensor.transpose(
                        pt[:], ws2[0:B, ts(c, P)], ident[:B, :B])
                else:
                    tr_i = nc.tensor.transpose(
                        pt[:], ws2[B:P, ts(c - KCE // 2, P)], identb[B:P, :])
                if c == 0:
                    tile.add_dep_helper(tr_i.ins, last_ermm.ins, sync=False,
                                        reason="head transposes after er matmuls on PE")
                nc.vector.tensor_copy(wsT[:, c, :], pt[:])
            pmh = ps_mm.tile([B, CH1C], F32, name="pmh", tag="pm")
            for c in range(KCE):
                nc.tensor.matmul(
                    pmh[:], wsT[:, c, :], ch1_sb[:, c, :],
                    start=(c == 0), stop=(c == KCE - 1),
                )
            if include_bias:
                nc.vector.tensor_add(pmh[:], pmh[:], chb1_sb[:])
            hh = acts.tile([B, CH1C], F16)
            nc.scalar.activation(hh[:], pmh[:], GELU)

            # ---- head GEMM2 (contraction shard): out_part = hh @ ch2_e ----
            hhT = acts.tile([P, 3, B], F16)
            for c in range(3):
                pt = ps_tr.tile([P, B], F16, name="pt", tag="pt")
                nc.tensor.transpose(pt[:], hh[:, ts(c, P)], ident[:B, :B])
                nc.vector.tensor_copy(hhT[:, c, :], pt[:])
            outsb = acts.tile([B, C], F32)
            for nn in range(2):
                pmo = ps_mm.tile([B, 512], F32, name="pmo", tag="pm")
                for c in range(3):
                    nc.tensor.matmul(
                        pmo[:, :500], hhT[:, c, :], ch2_sb[:, c, ds(nn * 500, 500)],
                        start=(c == 0), stop=(c == 2),
                    )
                nc.vector.tensor_copy(outsb[:, ds(nn * 500, 500)], pmo[:, :500])
                nc.sync.dma_start(outp[:, ds(nn * 500, 500)], outsb[:, ds(nn * 500, 500)])

    nc.finalize()
    return nc


_NC_CACHE: dict = {}


def _get_nc(include_bias: bool) -> bass.Bass:
    if include_bias not in _NC_CACHE:
        _NC_CACHE[include_bias] = _build(include_bias)
    return _NC_CACHE[include_bias]


def _pmajor(a, nchunk, dt):
    """[nchunk*128, cols] -> [128, nchunk, cols] partition-major, cast."""
    cols = a.shape[1]
    return np.ascontiguousarray(
        a.reshape(nchunk, P, cols).transpose(1, 0, 2)).astype(dt)


def _pack_inputs(inputs: dict, include_bias: bool) -> list[dict]:
    fp32 = np.float32
    x = np.ascontiguousarray(np.asarray(inputs["x"], dtype=fp32))  # (64,32,384)
    expert_emb = np.asarray(inputs["expert_emb"], dtype=fp32)      # (8,384)
    w1 = np.asarray(inputs["w1"], fp32)                            # (8,3072,3072)
    w2 = np.asarray(inputs["w2"], fp32)
    sw_w1 = np.asarray(inputs["sw_w1"], fp32)                      # (12288,12288)
    sw_w2 = np.asarray(inputs["sw_w2"], fp32)                      # (12288,8)
    ch_w1 = np.asarray(inputs["ch_w1"], fp32)                      # (3072,3072)
    ch_w2 = np.asarray(inputs["ch_w2"], fp32)                      # (3072,1000)

    x2 = x.reshape(NTOK, D)
    xt_base = x2.T.reshape(3, P, NTOK).transpose(1, 0, 2)          # (128,3,2048)
    x2h = (x2 / S1).astype(f16)                                    # (2048,384)
    xf = x.reshape(B, ND)
    xft_p = _pmajor(np.ascontiguousarray(xf.T), KCS, f16)          # (128,96,64)

    in_maps = []
    for e in range(NCORES):
        emb_p = expert_emb[e].reshape(3, P).T                      # (128,3)
        xt_p = np.ascontiguousarray(
            np.concatenate([xt_base, emb_p[:, :, None]], axis=2), dtype=fp32)
        sw1_e = np.ascontiguousarray(sw_w1[:, e * SWC:(e + 1) * SWC])
        sw1_p = _pmajor(sw1_e, KCS, f16)                           # (128,96,1536)
        w1_p = _pmajor(w1[e] * S1, KCE, e3m4)                      # (128,24,3072)
        w2p8 = _pmajor(w2[e][:, :W2F8C] * S2, KCE, e3m4)           # (128,24,2048)
        w2p16 = _pmajor(w2[e][:, W2F8C:], KCE, f16)                # (128,24,1024)
        sw2_e = np.ascontiguousarray(sw_w2[e * SWC:(e + 1) * SWC, :])
        sw2_p = _pmajor(sw2_e, KCH, f16)                           # (128,12,8)
        ch1_p = _pmajor(
            np.ascontiguousarray(ch_w1[:, e * CH1C:(e + 1) * CH1C]), KCE, f16)
        ch2_p = _pmajor(
            np.ascontiguousarray(ch_w2[e * CH1C:(e + 1) * CH1C, :]), 3, f16)
        m = {
            "xt": xt_p, "x2h": x2h, "xft": xft_p, "sw1p": sw1_p,
            "w1p": w1_p, "w2p8": w2p8, "w2p16": w2p16,
            "sw2p": sw2_p, "ch1p": ch1_p, "ch2p": ch2_p,
        }
        if include_bias:
            m["b1d"] = np.asarray(inputs["b1"][e], fp32).reshape(1, KD)
            m["b2d"] = np.asarray(inputs["b2"][e], fp32).reshape(1, KD)
            m["swb1d"] = np.asarray(
                inputs["sw_b1"], fp32).reshape(1, ND)[:, e * SWC:(e + 1) * SWC]
            m["swb2d"] = np.asarray(inputs["sw_b2"], fp32).reshape(1, E)
            m["chb1d"] = np.asarray(
                inputs["ch_b1"], fp32).reshape(1, KD)[:, e * CH1C:(e + 1) * CH1C]
        in_maps.append(m)
    return in_maps


def _need_bias(inputs) -> bool:
    return any(
        float(np.abs(np.asarray(inputs[k])).max()) != 0.0
        for k in ("b1", "b2", "sw_b1", "sw_b2", "ch_b1")
    )


def run(inputs: dict, **run_kwargs):
    """Run on the 8 cores; returns (full_output, BassKernelResults)."""
    include_bias = _need_bias(inputs)
    nc = _get_nc(include_bias)
    in_maps = _pack_inputs(inputs, include_bias)
    res = run_bass_kernel_spmd(nc, in_maps, core_ids=list(range(NCORES)), **run_kwargs)
    out = np.zeros((B, C), dtype=np.float64)
    for e in range(NCORES):
        out += res.results[e]["outp"].astype(np.float64)
    out += np.asarray(inputs["ch_b2"], np.float64)
    return out.astype(np.float32), res


def kernel(**inputs) -> np.ndarray:
    out, _ = run(inputs)
    return out


# revision 37
# speedup vs baseline: 1.1021x; 1.0048x over previous
"""Expert-choice MoE kernel for 8 Trainium2 NeuronCores (expert-parallel).

Decomposition (core e handles expert e):
  - router logits x . emb_e computed as bf16 hi/lo split (exact to ~2^-18)
    on PE; top-8 token indices per batch row via DVE max8/max_index; token
    gather via indirect DMA.
  - all activations/weights in fp16 (same bytes/speed as bf16, 8x less
    rounding noise), except: w1 fully in fp8-e3m4 and the first 2048 of
    w2's columns in fp8-e3m4 (scale 96, folded into x2h and the er
    copy-out respectively).
  - weights pre-packed host-side into partition-major [128, chunks, cols]
    layouts and streamed in 6-chunk groups (one DMA + a long matmul burst
    per group) so the PE stays busy and the DMA queue runs near the
    360 GB/s cap.
  - sum_weights GEMM1 column-sharded (each core owns 1536 columns of
    sw_w1).
  - ONE collective: AllGather of [er_e (64x3072) || z-partial (64x8)] in
    fp16. Cross-core wire DMAs are starved while the weight stream
    saturates the DMA engines, so pipelined AllReduces are useless here;
    a single post-stream AllGather whose input is only ready at stream
    end wastes nothing. Each core then sums z partials, softmaxes, and
    combines ws = sum_e wv[:,e]*er_e on DVE.
  - classification head sharded: GEMM1 column-shard (384 cols of ch_w1),
    GEMM2 contraction-shard (384 rows of ch_w2); per-core (64,1000)
    partials are summed on the host (+ ch_b2).
"""

import numpy as np
import ml_dtypes

import concourse.bass as bass
from concourse import bacc
import concourse.mybir as mybir
import concourse.tile as tile
from concourse.bass import ts, ds
from concourse.bass_utils import run_bass_kernel_spmd
from concourse.masks import make_identity

B, N, D, E, K, C = 64, 32, 384, 8, 8, 1000
KD, ND = K * D, N * D          # 3072, 12288
P = 128
NTOK = B * N                   # 2048
SWC = ND // E                  # 1536 sum-weights columns per core
CH1C = KD // E                 # 384 head-GEMM1 columns per core
KCE = KD // P                  # 24 k-chunks, expert GEMMs
KCS = ND // P                  # 96 k-chunks, sum-weights GEMM1
KCH = SWC // P                 # 12 k-chunks, z GEMM
NCORES = 8

G = 6                          # k-chunks per weight DMA group
W2F8C = 2048                   # fp8 columns of w2 (rest fp16)
S1 = 96.0                      # w1 fp8 scale (folded into x2h)
S2 = 96.0                      # w2 fp8 scale (folded into er copy-out)
AGW = KD + E                   # AllGather payload width (er || z)

F32 = mybir.dt.float32
F16 = mybir.dt.float16
BF16 = mybir.dt.bfloat16
F8E3 = mybir.dt.float8e3
U32 = mybir.dt.uint32
GELU = mybir.ActivationFunctionType.Gelu
EXP = mybir.ActivationFunctionType.Exp
X_AX = mybir.AxisListType.X
ADD = mybir.AluOpType.add
BYPASS = mybir.AluOpType.bypass
f16 = np.float16
bf16 = ml_dtypes.bfloat16
e3m4 = ml_dtypes.float8_e3m4


def _build(include_bias: bool) -> bass.Bass:
    nc = bacc.Bacc("TRN2", num_devices=NCORES)

    # xt: x.T in fp32 (router must match the reference's fp32 top-8 exactly;
    # a bf16 hi/lo split was ~8e-6 off and flipped one near-tie), emb_e
    # appended as column 2048
    xt = nc.dram_tensor("xt", [P, 3, NTOK + 1], F32, kind="ExternalInput")
    x2h = nc.dram_tensor("x2h", [NTOK, D], F16, kind="ExternalInput")
    xft = nc.dram_tensor("xft", [P, KCS, B], F16, kind="ExternalInput")
    sw1p = nc.dram_tensor("sw1p", [P, KCS, SWC], F16, kind="ExternalInput")
    w1p = nc.dram_tensor("w1p", [P, KCE, KD], F8E3, kind="ExternalInput")
    w2p8 = nc.dram_tensor("w2p8", [P, KCE, W2F8C], F8E3, kind="ExternalInput")
    w2p16 = nc.dram_tensor(
        "w2p16", [P, KCE, KD - W2F8C], F16, kind="ExternalInput")
    sw2p = nc.dram_tensor("sw2p", [P, KCH, E], F16, kind="ExternalInput")
    ch1p = nc.dram_tensor("ch1p", [P, KCE, CH1C], F16, kind="ExternalInput")
    ch2p = nc.dram_tensor("ch2p", [P, 3, C], F16, kind="ExternalInput")
    oh = nc.dram_tensor("oh", [B, E], F32, kind="ExternalInput")
    if include_bias:
        b1d = nc.dram_tensor("b1d", [1, KD], F32, kind="ExternalInput")
        b2d = nc.dram_tensor("b2d", [1, KD], F32, kind="ExternalInput")
        swb1d = nc.dram_tensor("swb1d", [1, SWC], F32, kind="ExternalInput")
        swb2d = nc.dram_tensor("swb2d", [1, E], F32, kind="ExternalInput")
        chb1d = nc.dram_tensor("chb1d", [1, CH1C], F32, kind="ExternalInput")
    outp = nc.dram_tensor("outp", [B, C], F32, kind="ExternalOutput")

    with tile.TileContext(nc) as tc:
        with (
            tc.tile_pool(name="consts", bufs=1) as consts,
            tc.tile_pool(name="acts", bufs=1) as acts,
            tc.tile_pool(name="wpool", bufs=4) as wpool,
            tc.tile_pool(name="ergp", bufs=4) as ergp,
            tc.tile_pool(name="ps_mm", bufs=6, space="PSUM") as ps_mm,
            tc.tile_pool(name="ps_tr", bufs=2, space="PSUM") as ps_tr,
            tc.tile_pool(name="dram", bufs=1, space="DRAM") as dram,
        ):
            # ---- constants / persistent activations ----
            ident = consts.tile([P, P], F16)
            make_identity(nc, ident[:])
            xft_sb = consts.tile([P, KCS, B], F16)
            nc.sync.dma_start(xft_sb[:], xft[:])
            sw2_sb = consts.tile([P, KCH, E], F16)
            nc.sync.dma_start(sw2_sb[:], sw2p[:])
            oh_sb = consts.tile([B, E], F32)
            nc.sync.dma_start(oh_sb[:], oh[:])
            pwarm = ps_tr.tile([P, B], F16, name="pwarm", tag="pt")
            nc.tensor.transpose(pwarm[:32, :32], ident[:32, :32], ident[:32, :32])
            if include_bias:
                b1_sb = consts.tile([B, KD], F32)
                nc.scalar.dma_start(b1_sb[:], b1d[0:1, :].to_broadcast([B, KD]))
                b2_sb = consts.tile([B, KD], F32)
                nc.scalar.dma_start(b2_sb[:], b2d[0:1, :].to_broadcast([B, KD]))
                swb1_sb = consts.tile([B, SWC], F32)
                nc.scalar.dma_start(swb1_sb[:], swb1d[0:1, :].to_broadcast([B, SWC]))
                swb2_sb = consts.tile([B, E], F32)
                nc.scalar.dma_start(swb2_sb[:], swb2d[0:1, :].to_broadcast([B, E]))
                chb1_sb = consts.tile([B, CH1C], F32)
                nc.scalar.dma_start(chb1_sb[:], chb1d[0:1, :].to_broadcast([B, CH1C]))

            # ---- phase 1: sum-weights GEMM1, 16 groups of 6 k-chunks ----
            pms = [ps_mm.tile([B, 512], F32, name=f"pms{n}", tag="pm")
                   for n in range(3)]

            def sw_group(g):
                wt = wpool.tile([P, G, SWC], F16, name="wt", tag="wt")
                nc.sync.dma_start(wt[:], sw1p[:, ts(g, G), :])
                for cg in range(G):
                    c = g * G + cg
                    for n in range(3):
                        nc.tensor.matmul(
                            pms[n][:], xft_sb[:, c, :], wt[:, cg, ts(n, 512)],
                            start=(c == 0), stop=(c == KCS - 1),
                        )

            # xt rides the scalar queue so the sw weight stream never pauses
            xt_sb = acts.tile([P, 3, NTOK + 1], F32)
            nc.scalar.dma_start(xt_sb[:], xt[:])
            sw_group(0)
            sw_group(1)

            # ---- router: logits = x @ emb_e, fp32 (emb packed as col 2048).
            # Emitted in quarters between sw groups: one contiguous 20us blob
            # of 4-cycle/row fp32 matmuls stalls the PE behind the stream.
            lg_flat = acts.tile([1, NTOK], F32)

            def router_quarter(nt):
                pr = ps_mm.tile([B, 512], F32, name="pr", tag="pm")
                for c in range(3):
                    nc.tensor.matmul(
                        pr[:1, :], xt_sb[:, c, NTOK : NTOK + 1],
                        xt_sb[:, c, ts(nt, 512)],
                        start=(c == 0), stop=(c == 2),
                    )
                nc.vector.tensor_copy(lg_flat[:, ts(nt, 512)], pr[:1, :])

            # ---- rest of the sw stream ----
            selT = acts.tile([P, KCE, B], F16)
            for g in range(2, KCS // G):
                sw_group(g)
                if 2 <= g <= 5:
                    router_quarter(g - 2)
            # ---- top-8 tokens per row + gather (overlaps sw streaming) ----
            lg_dram = dram.tile([1, NTOK], F32)
            nc.scalar.dma_start(lg_dram[:], lg_flat[:])
            lg_bn = acts.tile([B, N], F32)
            nc.scalar.dma_start(lg_bn[:], lg_dram[:].rearrange("x (b n) -> (x b) n", b=B))
            vals8 = acts.tile([B, 8], F32)
            idx8 = acts.tile([B, 8], U32)
            nc.vector.max(out=vals8[:], in_=lg_bn[:])
            nc.vector.max_index(out=idx8[:], in_max=vals8[:], in_values=lg_bn[:])
            base = acts.tile([B, 1], U32)
            nc.gpsimd.iota(base[:], pattern=[[0, 1]], base=0, channel_multiplier=N)
            off = acts.tile([B, 8], U32)
            nc.vector.tensor_tensor(
                out=off[:], in0=idx8[:], in1=base[:].to_broadcast([B, 8]), op=ADD
            )
            sel = acts.tile([B, K, D], F16)
            for k in range(K):
                nc.gpsimd.indirect_dma_start(
                    out=sel[:, k, :], out_offset=None,
                    in_=x2h[:],
                    in_offset=bass.IndirectOffsetOnAxis(ap=off[:, k : k + 1], axis=0),
                )
            sel_flat = sel[:].rearrange("b k d -> b (k d)")

            # selT transposes at the phase boundary: sel is long gathered, so
            # the PE never stalls on the router->top8->gather chain mid-stream
            for c in range(KCE):
                pt = ps_tr.tile([P, B], F16, name="pt", tag="pt")
                nc.tensor.transpose(pt[:], sel_flat[:, ts(c, P)], ident[:B, :B])
                nc.vector.tensor_copy(selT[:, c, :], pt[:])
            ch2_sb = consts.tile([P, 3, C], F16)
            nc.sync.dma_start(ch2_sb[:], ch2p[:])

            # ---- h1 = gelu(sw psums); z partial [64, 8] ----
            h1 = acts.tile([B, SWC], F16)
            for n in range(3):
                if include_bias:
                    nc.vector.tensor_add(pms[n][:], pms[n][:], swb1_sb[:, ts(n, 512)])
                nc.scalar.activation(h1[:, ts(n, 512)], pms[n][:], GELU)
            h1T = acts.tile([P, KCH, B], F16)
            for c in range(KCH):
                pt = ps_tr.tile([P, B], F16, name="pt", tag="pt")
                nc.tensor.transpose(pt[:], h1[:, ts(c, P)], ident[:B, :B])
                nc.vector.tensor_copy(h1T[:, c, :], pt[:])
            pz = ps_mm.tile([B, E], F32, name="pz", tag="pm")
            for c in range(KCH):
                nc.tensor.matmul(
                    pz[:], h1T[:, c, :], sw2_sb[:, c, :],
                    start=(c == 0), stop=(c == KCH - 1),
                )
            # z partials go in their own tiny early AllReduce; its wire DMAs
            # are starved while the weight stream runs, but it drains during
            # the stream's dips/end and is done before the big er AllGather
            # lands, so the softmax is off the critical path.
            z_sb = acts.tile([B, E], F32)
            nc.vector.tensor_copy(z_sb[:], pz[:])
            zin = dram.tile([B, E], F32)
            zout = dram.tile([B, E], F32)
            nc.gpsimd.dma_start(zin[:], z_sb[:])
            nc.gpsimd.collective_compute(
                "AllReduce", ADD, replica_groups=[list(range(NCORES))],
                ins=[zin[:].opt()], outs=[zout[:].opt()],
            )

            # ---- phase 2: expert GEMM1, fp8 w1 stream (4 groups of 6) ----
            h = acts.tile([B, KD], F16)
            pme = [ps_mm.tile([B, 512], F32, name=f"pme{n}", tag="pm")
                   for n in range(6)]
            for g in range(KCE // G):
                wt8 = wpool.tile([P, G, KD], F8E3, name="wt8", tag="wt")
                nc.sync.dma_start(wt8[:], w1p[:, ts(g, G), :])
                for cg in range(G):
                    c = g * G + cg
                    for n in range(6):
                        nc.tensor.matmul(
                            pme[n][:], selT[:, c, :], wt8[:, cg, ts(n, 512)],
                            start=(c == 0), stop=(c == KCE - 1),
                        )
            last_gelu = None
            for n in range(6):
                if include_bias:
                    nc.vector.tensor_add(pme[n][:], pme[n][:], b1_sb[:, ts(n, 512)])
                last_gelu = nc.scalar.activation(h[:, ts(n, 512)], pme[n][:], GELU)
            hT = acts.tile([P, KCE, B], F16)
            for c in range(KCE):
                pt = ps_tr.tile([P, B], F16, name="pt", tag="pt")
                nc.tensor.transpose(pt[:], h[:, ts(c, P)], ident[:B, :B])
                last_htc = nc.vector.tensor_copy(hT[:, c, :], pt[:])

            # ---- phase 3: expert GEMM2, single pass, 6 psum banks ----
            # cols 0:2048 fp8 (scale S2 undone at copy-out), 2048:3072 fp16
            ch1_sb = acts.tile([P, KCE, CH1C], F16)
            pm2 = [ps_mm.tile([B, 512], F32, name=f"pm2{n}", tag="pm")
                   for n in range(6)]
            last_ermm = None
            for g in range(KCE // G):
                wt2a = wpool.tile([P, G, W2F8C], F8E3, name="wt2a", tag="wt")
                nc.sync.dma_start(wt2a[:], w2p8[:, ts(g, G), :])
                wt2b = wpool.tile([P, G, KD - W2F8C], F16, name="wt2b", tag="wt")
                nc.sync.dma_start(wt2b[:], w2p16[:, ts(g, G), :])
                for cg in range(G):
                    c = g * G + cg
                    for n in range(6):
                        if n < 4:
                            rhs = wt2a[:, cg, ts(n, 512)]
                        else:
                            rhs = wt2b[:, cg, ts(n - 4, 512)]
                        last_ermm = nc.tensor.matmul(
                            pm2[n][:], hT[:, c, :], rhs,
                            start=(c == 0), stop=(c == KCE - 1),
                        )
                # stream the head GEMM1 weights behind the w2 groups
                if g < 3:
                    nc.sync.dma_start(ch1_sb[:, ts(g, 8), :], ch1p[:, ts(g, 8), :])
            # softmax over experts -> wv; we = wv[:, e] via the one-hot
            # (the program is identical on every core; per-core data only)
            zb = acts.tile([B, E], F32)
            zb_dma = nc.gpsimd.dma_start(zb[:], zout[:])
            if include_bias:
                nc.vector.tensor_add(zb[:], zb[:], swb2_sb[:])
            mx = acts.tile([B, 1], F32)
            mx_i = nc.vector.reduce_max(mx[:], zb[:], axis=X_AX)
            tile.add_dep_helper(mx_i.ins, last_htc.ins, sync=False,
                                reason="softmax after hT copies on DVE")
            nmx = acts.tile([B, 1], F32)
            nc.vector.tensor_scalar_mul(nmx[:], mx[:], -1.0)
            exps = acts.tile([B, E], F32)
            exp_i = nc.scalar.activation(exps[:], zb[:], EXP, bias=nmx[:])
            tile.add_dep_helper(exp_i.ins, last_gelu.ins, sync=False,
                                reason="Exp after expert gelus on ACT")
            gwarm = acts.tile([B, 1], F32)
            nc.scalar.activation(gwarm[:], mx[:], GELU)
            sm = acts.tile([B, 1], F32)
            nc.vector.reduce_sum(sm[:], exps[:], axis=X_AX)
            rs = acts.tile([B, 1], F32)
            nc.vector.reciprocal(rs[:], sm[:])
            wv = acts.tile([B, E], F32)
            nc.vector.tensor_scalar_mul(wv[:], exps[:], rs[:])
            t8 = acts.tile([B, E], F32)
            nc.vector.tensor_mul(out=t8[:], in0=wv[:], in1=oh_sb[:])
            we = acts.tile([B, 1], F32)
            nc.vector.reduce_sum(we[:], t8[:], axis=X_AX)
            we8 = acts.tile([B, 1], F32)
            nc.vector.tensor_scalar_mul(we8[:], we[:], 1.0 / S2)

            # er copy-out scaled by we (and 1/S2 on the fp8 columns); the
            # AllReduce then IS the expert combine -- no gather, no local sum
            er_sb = acts.tile([B, KD], F16)
            for n in range(6):
                wej = we8 if n < 4 else we
                if include_bias:
                    stage = acts.tile([B, 512], F32, name=f"stage{n}")
                    sc = (1.0 / S2) if n < 4 else 1.0
                    nc.vector.tensor_scalar_mul(stage[:], pm2[n][:], sc)
                    nc.vector.tensor_add(stage[:], stage[:], b2_sb[:, ts(n, 512)])
                    nc.vector.tensor_scalar_mul(stage[:], stage[:], we[:])
                    nc.vector.tensor_copy(er_sb[:, ts(n, 512)], stage[:])
                else:
                    nc.vector.tensor_scalar_mul(
                        er_sb[:, ts(n, 512)], pm2[n][:], wej[:])
            cin = dram.tile([B, KD], F16)
            wsout = dram.tile([B, KD], F16)
            cin_dma = nc.scalar.dma_start(cin[:], er_sb[:])
            nc.gpsimd.collective_compute(
                "AllReduce", ADD, replica_groups=[list(range(NCORES))],
                ins=[cin[:].opt()], outs=[wsout[:].opt()],
            )
            ws = acts.tile([B, KD], F16)
            ws_dma = nc.scalar.dma_start(ws[:], wsout[:])
            tile.add_dep_helper(ws_dma.ins, cin_dma.ins, sync=False,
                                reason="ws readback after cin send on scalar q")

            # ---- head GEMM1 (column shard): hh = gelu(ws @ ch1_e) ----
            # transposes first (PE pipelines with the DVE copies), then the
            # matmul burst back-to-back — avoids 24 PE<->DVE sem round trips.
            wsT = acts.tile([P, KCE, B], F16)
            for c in range(KCE):
                pt = ps_tr.tile([P, B], F16, name="pt", tag="pt")
                tr_i = nc.tensor.transpose(pt[:], ws[:, ts(c, P)], ident[:B, :B])
                if c == 0:
                    tile.add_dep_helper(tr_i.ins, last_ermm.ins, sync=False,
                                        reason="head transposes after er matmuls on PE")
                nc.vector.tensor_copy(wsT[:, c, :], pt[:])
            pmh = ps_mm.tile([B, CH1C], F32, name="pmh", tag="pm")
            for c in range(KCE):
                nc.tensor.matmul(
                    pmh[:], wsT[:, c, :], ch1_sb[:, c, :],
                    start=(c == 0), stop=(c == KCE - 1),
                )
            if include_bias:
                nc.vector.tensor_add(pmh[:], pmh[:], chb1_sb[:])
            hh = acts.tile([B, CH1C], F16)
            nc.scalar.activation(hh[:], pmh[:], GELU)

            # ---- head GEMM2 (contraction shard): out_part = hh @ ch2_e ----
            hhT = acts.tile([P, 3, B], F16)
            for c in range(3):
                pt = ps_tr.tile([P, B], F16, name="pt", tag="pt")
                nc.tensor.transpose(pt[:], hh[:, ts(c, P)], ident[:B, :B])
                nc.vector.tensor_copy(hhT[:, c, :], pt[:])
            outsb = acts.tile([B, C], F32)
            for nn in range(2):
                pmo = ps_mm.tile([B, 512], F32, name="pmo", tag="pm")
                for c in range(3):
                    nc.tensor.matmul(
                        pmo[:, :500], hhT[:, c, :], ch2_sb[:, c, ds(nn * 500, 500)],
                        start=(c == 0), stop=(c == 2),
                    )
                nc.vector.tensor_copy(outsb[:, ds(nn * 500, 500)], pmo[:, :500])
                nc.sync.dma_start(outp[:, ds(nn * 500, 500)], outsb[:, ds(nn * 500, 500)])

    nc.finalize()
    return nc


_NC_CACHE: dict = {}


def _get_nc(include_bias: bool) -> bass.Bass:
    if include_bias not in _NC_CACHE:
        _NC_CACHE[include_bias] = _build(include_bias)
    return _NC_CACHE[include_bias]


def _pmajor(a, nchunk, dt):
    """[nchunk*128, cols] -> [128, nchunk, cols] partition-major, cast."""
    cols = a.shape[1]
    return np.ascontiguousarray(
        a.reshape(nchunk, P, cols).transpose(1, 0, 2)).astype(dt)


def _pack_inputs(inputs: dict, include_bias: bool) -> list[dict]:
    fp32 = np.float32
    x = np.ascontiguousarray(np.asarray(inputs["x"], dtype=fp32))  # (64,32,384)
    expert_emb = np.asarray(inputs["expert_emb"], dtype=fp32)      # (8,384)
    w1 = np.asarray(inputs["w1"], fp32)                            # (8,3072,3072)
    w2 = np.asarray(inputs["w2"], fp32)
    sw_w1 = np.asarray(inputs["sw_w1"], fp32)                      # (12288,12288)
    sw_w2 = np.asarray(inputs["sw_w2"], fp32)                      # (12288,8)
    ch_w1 = np.asarray(inputs["ch_w1"], fp32)                      # (3072,3072)
    ch_w2 = np.asarray(inputs["ch_w2"], fp32)                      # (3072,1000)

    x2 = x.reshape(NTOK, D)
    xt_base = x2.T.reshape(3, P, NTOK).transpose(1, 0, 2)          # (128,3,2048)
    x2h = (x2 / S1).astype(f16)                                    # (2048,384)
    xf = x.reshape(B, ND)
    xft_p = _pmajor(np.ascontiguousarray(xf.T), KCS, f16)          # (128,96,64)

    in_maps = []
    for e in range(NCORES):
        emb_p = expert_emb[e].reshape(3, P).T                      # (128,3)
        xt_p = np.ascontiguousarray(
            np.concatenate([xt_base, emb_p[:, :, None]], axis=2), dtype=fp32)
        sw1_e = np.ascontiguousarray(sw_w1[:, e * SWC:(e + 1) * SWC])
        sw1_p = _pmajor(sw1_e, KCS, f16)                           # (128,96,1536)
        w1_p = _pmajor(w1[e] * S1, KCE, e3m4)                      # (128,24,3072)
        w2p8 = _pmajor(w2[e][:, :W2F8C] * S2, KCE, e3m4)           # (128,24,2048)
        w2p16 = _pmajor(w2[e][:, W2F8C:], KCE, f16)                # (128,24,1024)
        sw2_e = np.ascontiguousarray(sw_w2[e * SWC:(e + 1) * SWC, :])
        sw2_p = _pmajor(sw2_e, KCH, f16)                           # (128,12,8)
        ch1_p = _pmajor(
            np.ascontiguousarray(ch_w1[:, e * CH1C:(e + 1) * CH1C]), KCE, f16)
        ch2_p = _pmajor(
            np.ascontiguousarray(ch_w2[e * CH1C:(e + 1) * CH1C, :]), 3, f16)
        oh_p = np.zeros((B, E), dtype=fp32)
        oh_p[:, e] = 1.0
        m = {
            "xt": xt_p, "x2h": x2h, "xft": xft_p, "sw1p": sw1_p, "oh": oh_p,
            "w1p": w1_p, "w2p8": w2p8, "w2p16": w2p16,
            "sw2p": sw2_p, "ch1p": ch1_p, "ch2p": ch2_p,
        }
        if include_bias:
            m["b1d"] = np.asarray(inputs["b1"][e], fp32).reshape(1, KD)
            m["b2d"] = np.asarray(inputs["b2"][e], fp32).reshape(1, KD)
            m["swb1d"] = np.asarray(
                inputs["sw_b1"], fp32).reshape(1, ND)[:, e * SWC:(e + 1) * SWC]
            m["swb2d"] = np.asarray(inputs["sw_b2"], fp32).reshape(1, E)
            m["chb1d"] = np.asarray(
                inputs["ch_b1"], fp32).reshape(1, KD)[:, e * CH1C:(e + 1) * CH1C]
        in_maps.append(m)
    return in_maps


def _need_bias(inputs) -> bool:
    return any(
        float(np.abs(np.asarray(inputs[k])).max()) != 0.0
        for k in ("b1", "b2", "sw_b1", "sw_b2", "ch_b1")
    )


def run(inputs: dict, **run_kwargs):
    """Run on the 8 cores; returns (full_output, BassKernelResults)."""
    include_bias = _need_bias(inputs)
    nc = _get_nc(include_bias)
    in_maps = _pack_inputs(inputs, include_bias)
    res = run_bass_kernel_spmd(nc, in_maps, core_ids=list(range(NCORES)), **run_kwargs)
    out = np.zeros((B, C), dtype=np.float64)
    for e in range(NCORES):
        out += res.results[e]["outp"].astype(np.float64)
    out += np.asarray(inputs["ch_b2"], np.float64)
    return out.astype(np.float32), res


def kernel(**inputs) -> np.ndarray:
    out, _ = run(inputs)
    return out


# revision 39
# speedup vs baseline: 1.1480x; 1.0417x over previous
"""Expert-choice MoE kernel for 8 Trainium2 NeuronCores (expert-parallel).

Decomposition (core e handles expert e):
  - router logits x . emb_e computed as bf16 hi/lo split (exact to ~2^-18)
    on PE; top-8 token indices per batch row via DVE max8/max_index; token
    gather via indirect DMA.
  - all activations/weights in fp16 (same bytes/speed as bf16, 8x less
    rounding noise), except: w1 fully in fp8-e3m4 and the first 2048 of
    w2's columns in fp8-e3m4 (scale 96, folded into x2h and the er
    copy-out respectively).
  - weights pre-packed host-side into partition-major [128, chunks, cols]
    layouts and streamed in 6-chunk groups (one DMA + a long matmul burst
    per group) so the PE stays busy and the DMA queue runs near the
    360 GB/s cap.
  - sum_weights GEMM1 column-sharded (each core owns 1536 columns of
    sw_w1).
  - ONE collective: AllGather of [er_e (64x3072) || z-partial (64x8)] in
    fp16. Cross-core wire DMAs are starved while the weight stream
    saturates the DMA engines, so pipelined AllReduces are useless here;
    a single post-stream AllGather whose input is only ready at stream
    end wastes nothing. Each core then sums z partials, softmaxes, and
    combines ws = sum_e wv[:,e]*er_e on DVE.
  - classification head sharded: GEMM1 column-shard (384 cols of ch_w1),
    GEMM2 contraction-shard (384 rows of ch_w2); per-core (64,1000)
    partials are summed on the host (+ ch_b2).
"""

import numpy as np
import ml_dtypes

import concourse.bass as bass
from concourse import bacc
import concourse.mybir as mybir
import concourse.tile as tile
from concourse.bass import ts, ds
from concourse.bass_utils import run_bass_kernel_spmd
from concourse.masks import make_identity

B, N, D, E, K, C = 64, 32, 384, 8, 8, 1000
KD, ND = K * D, N * D          # 3072, 12288
P = 128
NTOK = B * N                   # 2048
SWC = ND // E                  # 1536 sum-weights columns per core
CH1C = KD // E                 # 384 head-GEMM1 columns per core
KCE = KD // P                  # 24 k-chunks, expert GEMMs
KCS = ND // P                  # 96 k-chunks, sum-weights GEMM1
KCH = SWC // P                 # 12 k-chunks, z GEMM
NCORES = 8

G = 6                          # k-chunks per weight DMA group
W2F8C = 2048                   # fp8 columns of w2 (rest fp16)
S1 = 96.0                      # w1 fp8 scale (folded into x2h)
S2 = 96.0                      # w2 fp8 scale (folded into er copy-out)
AGW = KD + E                   # AllGather payload width (er || z)

F32 = mybir.dt.float32
F16 = mybir.dt.float16
BF16 = mybir.dt.bfloat16
F8E3 = mybir.dt.float8e3
U32 = mybir.dt.uint32
GELU = mybir.ActivationFunctionType.Gelu
EXP = mybir.ActivationFunctionType.Exp
X_AX = mybir.AxisListType.X
ADD = mybir.AluOpType.add
BYPASS = mybir.AluOpType.bypass
f16 = np.float16
bf16 = ml_dtypes.bfloat16
e3m4 = ml_dtypes.float8_e3m4


def _build(include_bias: bool) -> bass.Bass:
    nc = bacc.Bacc("TRN2", num_devices=NCORES)

    # Router inputs, fp32 (the router must match the reference's fp32 top-8
    # exactly; a bf16 hi/lo split was ~8e-6 off and flipped one near-tie).
    # The dot products run on the idle DVE: every PE cycle is precious (the
    # PE sits at mid p-state and is the mid-section bottleneck).
    # xp[2b+h, m, :] = x[b, 16h+m, :]; embq = emb_e tiled 4x.
    xp = nc.dram_tensor("xp", [P, N // 2, D], F32, kind="ExternalInput")
    embq = nc.dram_tensor("embq", [1, 4 * D], F32, kind="ExternalInput")
    x2h = nc.dram_tensor("x2h", [NTOK, D], F16, kind="ExternalInput")
    xft = nc.dram_tensor("xft", [P, KCS, B], F16, kind="ExternalInput")
    sw1p = nc.dram_tensor("sw1p", [P, KCS, SWC], F16, kind="ExternalInput")
    w1p = nc.dram_tensor("w1p", [P, KCE, KD], F8E3, kind="ExternalInput")
    w2p8 = nc.dram_tensor("w2p8", [P, KCE, W2F8C], F8E3, kind="ExternalInput")
    w2p16 = nc.dram_tensor(
        "w2p16", [P, KCE, KD - W2F8C], F16, kind="ExternalInput")
    sw2p = nc.dram_tensor("sw2p", [P, KCH, E], F16, kind="ExternalInput")
    ch1p = nc.dram_tensor("ch1p", [P, KCE, CH1C], F16, kind="ExternalInput")
    ch2p = nc.dram_tensor("ch2p", [P, 3, C], F16, kind="ExternalInput")
    oh = nc.dram_tensor("oh", [B, E], F32, kind="ExternalInput")
    if include_bias:
        b1d = nc.dram_tensor("b1d", [1, KD], F32, kind="ExternalInput")
        b2d = nc.dram_tensor("b2d", [1, KD], F32, kind="ExternalInput")
        swb1d = nc.dram_tensor("swb1d", [1, SWC], F32, kind="ExternalInput")
        swb2d = nc.dram_tensor("swb2d", [1, E], F32, kind="ExternalInput")
        chb1d = nc.dram_tensor("chb1d", [1, CH1C], F32, kind="ExternalInput")
    outp = nc.dram_tensor("outp", [B, C], F32, kind="ExternalOutput")

    with tile.TileContext(nc) as tc:
        with (
            tc.tile_pool(name="consts", bufs=1) as consts,
            tc.tile_pool(name="acts", bufs=1) as acts,
            tc.tile_pool(name="wpool", bufs=4) as wpool,
            tc.tile_pool(name="ergp", bufs=4) as ergp,
            tc.tile_pool(name="ps_mm", bufs=6, space="PSUM") as ps_mm,
            tc.tile_pool(name="ps_tr", bufs=2, space="PSUM") as ps_tr,
            tc.tile_pool(name="dram", bufs=1, space="DRAM") as dram,
        ):
            # ---- constants / persistent activations ----
            ident = consts.tile([P, P], F16)
            make_identity(nc, ident[:])
            xft_sb = consts.tile([P, KCS, B], F16)
            nc.sync.dma_start(xft_sb[:], xft[:])
            sw2_sb = consts.tile([P, KCH, E], F16)
            nc.sync.dma_start(sw2_sb[:], sw2p[:])
            oh_sb = consts.tile([B, E], F32)
            nc.sync.dma_start(oh_sb[:], oh[:])
            pwarm = ps_tr.tile([P, B], F16, name="pwarm", tag="pt")
            nc.tensor.transpose(pwarm[:32, :32], ident[:32, :32], ident[:32, :32])
            if include_bias:
                b1_sb = consts.tile([B, KD], F32)
                nc.scalar.dma_start(b1_sb[:], b1d[0:1, :].to_broadcast([B, KD]))
                b2_sb = consts.tile([B, KD], F32)
                nc.scalar.dma_start(b2_sb[:], b2d[0:1, :].to_broadcast([B, KD]))
                swb1_sb = consts.tile([B, SWC], F32)
                nc.scalar.dma_start(swb1_sb[:], swb1d[0:1, :].to_broadcast([B, SWC]))
                swb2_sb = consts.tile([B, E], F32)
                nc.scalar.dma_start(swb2_sb[:], swb2d[0:1, :].to_broadcast([B, E]))
                chb1_sb = consts.tile([B, CH1C], F32)
                nc.scalar.dma_start(chb1_sb[:], chb1d[0:1, :].to_broadcast([B, CH1C]))

            # ---- phase 1: sum-weights GEMM1, 16 groups of 6 k-chunks ----
            pms = [ps_mm.tile([B, 512], F32, name=f"pms{n}", tag="pm")
                   for n in range(3)]

            def sw_group(g):
                wt = wpool.tile([P, G, SWC], F16, name="wt", tag="wt")
                nc.sync.dma_start(wt[:], sw1p[:, ts(g, G), :])
                for cg in range(G):
                    c = g * G + cg
                    for n in range(3):
                        nc.tensor.matmul(
                            pms[n][:], xft_sb[:, c, :], wt[:, cg, ts(n, 512)],
                            start=(c == 0), stop=(c == KCS - 1),
                        )

            # router inputs ride the scalar queue; sw stream never pauses
            xp_sb = acts.tile([P, N // 2, D], F32)
            nc.scalar.dma_start(xp_sb[:], xp[:])
            embq_sb = acts.tile([P, 4 * D], F32)
            nc.scalar.dma_start(embq_sb[:], embq[0:1, :].to_broadcast([P, 4 * D]))
            sw_group(0)
            sw_group(1)

            # ---- router on DVE: logits[p, m] = x[p, m, :] . emb, fp32 ----
            lg_p = acts.tile([P, N // 2], F32)
            rt = acts.tile([P, 4 * D], F32)

            def router_quarter(nt):
                xs = xp_sb[:, ds(nt * 4, 4), :].rearrange("p m d -> p (m d)")
                nc.vector.tensor_mul(out=rt[:], in0=xs, in1=embq_sb[:])
                nc.vector.reduce_sum(
                    lg_p[:, ds(nt * 4, 4)],
                    rt[:].rearrange("p (m d) -> p m d", d=D), axis=X_AX)

            # ---- rest of the sw stream ----
            selT = acts.tile([P, KCE, B], F16)
            for g in range(2, KCS // G):
                sw_group(g)
                if 2 <= g <= 5:
                    router_quarter(g - 2)
            # ---- top-8 tokens per row + gather (overlaps sw streaming) ----
            lg_dram = dram.tile([P, N // 2], F32)
            nc.scalar.dma_start(lg_dram[:], lg_p[:])
            lg_bn = acts.tile([B, N], F32)
            nc.scalar.dma_start(
                lg_bn[:], lg_dram[:].rearrange("(b h) j -> b (h j)", h=2))
            vals8 = acts.tile([B, 8], F32)
            idx8 = acts.tile([B, 8], U32)
            nc.vector.max(out=vals8[:], in_=lg_bn[:])
            nc.vector.max_index(out=idx8[:], in_max=vals8[:], in_values=lg_bn[:])
            base = acts.tile([B, 1], U32)
            nc.gpsimd.iota(base[:], pattern=[[0, 1]], base=0, channel_multiplier=N)
            off = acts.tile([B, 8], U32)
            nc.vector.tensor_tensor(
                out=off[:], in0=idx8[:], in1=base[:].to_broadcast([B, 8]), op=ADD
            )
            sel = acts.tile([B, K, D], F16)
            for k in range(K):
                nc.gpsimd.indirect_dma_start(
                    out=sel[:, k, :], out_offset=None,
                    in_=x2h[:],
                    in_offset=bass.IndirectOffsetOnAxis(ap=off[:, k : k + 1], axis=0),
                )
            sel_flat = sel[:].rearrange("b k d -> b (k d)")

            # selT transposes at the phase boundary: sel is long gathered, so
            # the PE never stalls on the router->top8->gather chain mid-stream
            for c in range(KCE):
                pt = ps_tr.tile([P, B], F16, name="pt", tag="pt")
                nc.tensor.transpose(pt[:], sel_flat[:, ts(c, P)], ident[:B, :B])
                nc.vector.tensor_copy(selT[:, c, :], pt[:])
            ch2_sb = consts.tile([P, 3, C], F16)
            nc.sync.dma_start(ch2_sb[:], ch2p[:])

            # ---- h1 = gelu(sw psums); z partial [64, 8] ----
            h1 = acts.tile([B, SWC], F16)
            for n in range(3):
                if include_bias:
                    nc.vector.tensor_add(pms[n][:], pms[n][:], swb1_sb[:, ts(n, 512)])
                nc.scalar.activation(h1[:, ts(n, 512)], pms[n][:], GELU)
            h1T = acts.tile([P, KCH, B], F16)
            for c in range(KCH):
                pt = ps_tr.tile([P, B], F16, name="pt", tag="pt")
                nc.tensor.transpose(pt[:], h1[:, ts(c, P)], ident[:B, :B])
                nc.vector.tensor_copy(h1T[:, c, :], pt[:])
            pz = ps_mm.tile([B, E], F32, name="pz", tag="pm")
            for c in range(KCH):
                nc.tensor.matmul(
                    pz[:], h1T[:, c, :], sw2_sb[:, c, :],
                    start=(c == 0), stop=(c == KCH - 1),
                )
            # z partials go in their own tiny early AllReduce; its wire DMAs
            # are starved while the weight stream runs, but it drains during
            # the stream's dips/end and is done before the big er AllGather
            # lands, so the softmax is off the critical path.
            z_sb = acts.tile([B, E], F32)
            nc.vector.tensor_copy(z_sb[:], pz[:])
            zin = dram.tile([B, E], F32)
            zout = dram.tile([B, E], F32)
            nc.gpsimd.dma_start(zin[:], z_sb[:])
            nc.gpsimd.collective_compute(
                "AllReduce", ADD, replica_groups=[list(range(NCORES))],
                ins=[zin[:].opt()], outs=[zout[:].opt()],
            )

            # ---- phase 2: expert GEMM1, fp8 w1 stream (4 groups of 6) ----
            h = acts.tile([B, KD], F16)
            pme = [ps_mm.tile([B, 512], F32, name=f"pme{n}", tag="pm")
                   for n in range(6)]
            for g in range(KCE // G):
                wt8 = wpool.tile([P, G, KD], F8E3, name="wt8", tag="wt")
                nc.sync.dma_start(wt8[:], w1p[:, ts(g, G), :])
                for cg in range(G):
                    c = g * G + cg
                    for n in range(6):
                        nc.tensor.matmul(
                            pme[n][:], selT[:, c, :], wt8[:, cg, ts(n, 512)],
                            start=(c == 0), stop=(c == KCE - 1),
                        )
            last_gelu = None
            for n in range(6):
                if include_bias:
                    nc.vector.tensor_add(pme[n][:], pme[n][:], b1_sb[:, ts(n, 512)])
                last_gelu = nc.scalar.activation(h[:, ts(n, 512)], pme[n][:], GELU)
            hT = acts.tile([P, KCE, B], F16)
            for c in range(KCE):
                pt = ps_tr.tile([P, B], F16, name="pt", tag="pt")
                nc.tensor.transpose(pt[:], h[:, ts(c, P)], ident[:B, :B])
                last_htc = nc.vector.tensor_copy(hT[:, c, :], pt[:])

            # ---- phase 3: expert GEMM2, single pass, 6 psum banks ----
            # cols 0:2048 fp8 (scale S2 undone at copy-out), 2048:3072 fp16
            ch1_sb = acts.tile([P, KCE, CH1C], F16)
            pm2 = [ps_mm.tile([B, 512], F32, name=f"pm2{n}", tag="pm")
                   for n in range(6)]
            last_ermm = None
            for g in range(KCE // G):
                wt2a = wpool.tile([P, G, W2F8C], F8E3, name="wt2a", tag="wt")
                nc.sync.dma_start(wt2a[:], w2p8[:, ts(g, G), :])
                wt2b = wpool.tile([P, G, KD - W2F8C], F16, name="wt2b", tag="wt")
                nc.sync.dma_start(wt2b[:], w2p16[:, ts(g, G), :])
                for cg in range(G):
                    c = g * G + cg
                    for n in range(6):
                        if n < 4:
                            rhs = wt2a[:, cg, ts(n, 512)]
                        else:
                            rhs = wt2b[:, cg, ts(n - 4, 512)]
                        last_ermm = nc.tensor.matmul(
                            pm2[n][:], hT[:, c, :], rhs,
                            start=(c == 0), stop=(c == KCE - 1),
                        )
                # stream the head GEMM1 weights behind the w2 groups
                if g < 3:
                    nc.sync.dma_start(ch1_sb[:, ts(g, 8), :], ch1p[:, ts(g, 8), :])
            # softmax over experts -> wv; we = wv[:, e] via the one-hot
            # (the program is identical on every core; per-core data only)
            zb = acts.tile([B, E], F32)
            zb_dma = nc.gpsimd.dma_start(zb[:], zout[:])
            if include_bias:
                nc.vector.tensor_add(zb[:], zb[:], swb2_sb[:])
            mx = acts.tile([B, 1], F32)
            mx_i = nc.vector.reduce_max(mx[:], zb[:], axis=X_AX)
            tile.add_dep_helper(mx_i.ins, last_htc.ins, sync=False,
                                reason="softmax after hT copies on DVE")
            nmx = acts.tile([B, 1], F32)
            nc.vector.tensor_scalar_mul(nmx[:], mx[:], -1.0)
            exps = acts.tile([B, E], F32)
            exp_i = nc.scalar.activation(exps[:], zb[:], EXP, bias=nmx[:])
            tile.add_dep_helper(exp_i.ins, last_gelu.ins, sync=False,
                                reason="Exp after expert gelus on ACT")
            gwarm = acts.tile([B, 1], F32)
            nc.scalar.activation(gwarm[:], mx[:], GELU)
            sm = acts.tile([B, 1], F32)
            nc.vector.reduce_sum(sm[:], exps[:], axis=X_AX)
            rs = acts.tile([B, 1], F32)
            nc.vector.reciprocal(rs[:], sm[:])
            wv = acts.tile([B, E], F32)
            nc.vector.tensor_scalar_mul(wv[:], exps[:], rs[:])
            t8 = acts.tile([B, E], F32)
            nc.vector.tensor_mul(out=t8[:], in0=wv[:], in1=oh_sb[:])
            we = acts.tile([B, 1], F32)
            nc.vector.reduce_sum(we[:], t8[:], axis=X_AX)
            we8 = acts.tile([B, 1], F32)
            nc.vector.tensor_scalar_mul(we8[:], we[:], 1.0 / S2)

            # er copy-out scaled by we (and 1/S2 on the fp8 columns); the
            # AllReduce then IS the expert combine -- no gather, no local sum
            er_sb = acts.tile([B, KD], F16)
            for n in range(6):
                wej = we8 if n < 4 else we
                if include_bias:
                    stage = acts.tile([B, 512], F32, name=f"stage{n}")
                    sc = (1.0 / S2) if n < 4 else 1.0
                    nc.vector.tensor_scalar_mul(stage[:], pm2[n][:], sc)
                    nc.vector.tensor_add(stage[:], stage[:], b2_sb[:, ts(n, 512)])
                    nc.vector.tensor_scalar_mul(stage[:], stage[:], we[:])
                    nc.vector.tensor_copy(er_sb[:, ts(n, 512)], stage[:])
                else:
                    nc.vector.tensor_scalar_mul(
                        er_sb[:, ts(n, 512)], pm2[n][:], wej[:])
            cin = dram.tile([B, KD], F16)
            wsout = dram.tile([B, KD], F16)
            cin_dma = nc.scalar.dma_start(cin[:], er_sb[:])
            nc.gpsimd.collective_compute(
                "AllReduce", ADD, replica_groups=[list(range(NCORES))],
                ins=[cin[:].opt()], outs=[wsout[:].opt()],
            )
            ws = acts.tile([B, KD], F16)
            ws_dma = nc.scalar.dma_start(ws[:], wsout[:])
            tile.add_dep_helper(ws_dma.ins, cin_dma.ins, sync=False,
                                reason="ws readback after cin send on scalar q")

            # ---- head GEMM1 (column shard): hh = gelu(ws @ ch1_e) ----
            # transposes first (PE pipelines with the DVE copies), then the
            # matmul burst back-to-back — avoids 24 PE<->DVE sem round trips.
            wsT = acts.tile([P, KCE, B], F16)
            for c in range(KCE):
                pt = ps_tr.tile([P, B], F16, name="pt", tag="pt")
                tr_i = nc.tensor.transpose(pt[:], ws[:, ts(c, P)], ident[:B, :B])
                if c == 0:
                    tile.add_dep_helper(tr_i.ins, last_ermm.ins, sync=False,
                                        reason="head transposes after er matmuls on PE")
                nc.vector.tensor_copy(wsT[:, c, :], pt[:])
            pmh = ps_mm.tile([B, CH1C], F32, name="pmh", tag="pm")
            for c in range(KCE):
                nc.tensor.matmul(
                    pmh[:], wsT[:, c, :], ch1_sb[:, c, :],
                    start=(c == 0), stop=(c == KCE - 1),
                )
            if include_bias:
                nc.vector.tensor_add(pmh[:], pmh[:], chb1_sb[:])
            hh = acts.tile([B, CH1C], F16)
            nc.scalar.activation(hh[:], pmh[:], GELU)

            # ---- head GEMM2 (contraction shard): out_part = hh @ ch2_e ----
            hhT = acts.tile([P, 3, B], F16)
            for c in range(3):
                pt = ps_tr.tile([P, B], F16, name="pt", tag="pt")
                nc.tensor.transpose(pt[:], hh[:, ts(c, P)], ident[:B, :B])
                nc.vector.tensor_copy(hhT[:, c, :], pt[:])
            outsb = acts.tile([B, C], F32)
            for nn in range(2):
                pmo = ps_mm.tile([B, 512], F32, name="pmo", tag="pm")
                for c in range(3):
                    nc.tensor.matmul(
                        pmo[:, :500], hhT[:, c, :], ch2_sb[:, c, ds(nn * 500, 500)],
                        start=(c == 0), stop=(c == 2),
                    )
                nc.vector.tensor_copy(outsb[:, ds(nn * 500, 500)], pmo[:, :500])
                nc.sync.dma_start(outp[:, ds(nn * 500, 500)], outsb[:, ds(nn * 500, 500)])

    nc.finalize()
    return nc


_NC_CACHE: dict = {}


def _get_nc(include_bias: bool) -> bass.Bass:
    if include_bias not in _NC_CACHE:
        _NC_CACHE[include_bias] = _build(include_bias)
    return _NC_CACHE[include_bias]


def _pmajor(a, nchunk, dt):
    """[nchunk*128, cols] -> [128, nchunk, cols] partition-major, cast."""
    cols = a.shape[1]
    return np.ascontiguousarray(
        a.reshape(nchunk, P, cols).transpose(1, 0, 2)).astype(dt)


def _pack_inputs(inputs: dict, include_bias: bool) -> list[dict]:
    fp32 = np.float32
    x = np.ascontiguousarray(np.asarray(inputs["x"], dtype=fp32))  # (64,32,384)
    expert_emb = np.asarray(inputs["expert_emb"], dtype=fp32)      # (8,384)
    w1 = np.asarray(inputs["w1"], fp32)                            # (8,3072,3072)
    w2 = np.asarray(inputs["w2"], fp32)
    sw_w1 = np.asarray(inputs["sw_w1"], fp32)                      # (12288,12288)
    sw_w2 = np.asarray(inputs["sw_w2"], fp32)                      # (12288,8)
    ch_w1 = np.asarray(inputs["ch_w1"], fp32)                      # (3072,3072)
    ch_w2 = np.asarray(inputs["ch_w2"], fp32)                      # (3072,1000)

    x2 = x.reshape(NTOK, D)
    xp_p = np.ascontiguousarray(
        x.reshape(B, 2, N // 2, D).reshape(P, N // 2, D))          # (128,16,384)
    x2h = (x2 / S1).astype(f16)                                    # (2048,384)
    xf = x.reshape(B, ND)
    xft_p = _pmajor(np.ascontiguousarray(xf.T), KCS, f16)          # (128,96,64)

    in_maps = []
    for e in range(NCORES):
        embq_p = np.tile(expert_emb[e], 4).reshape(1, 4 * D).astype(fp32)
        sw1_e = np.ascontiguousarray(sw_w1[:, e * SWC:(e + 1) * SWC])
        sw1_p = _pmajor(sw1_e, KCS, f16)                           # (128,96,1536)
        w1_p = _pmajor(w1[e] * S1, KCE, e3m4)                      # (128,24,3072)
        w2p8 = _pmajor(w2[e][:, :W2F8C] * S2, KCE, e3m4)           # (128,24,2048)
        w2p16 = _pmajor(w2[e][:, W2F8C:], KCE, f16)                # (128,24,1024)
        sw2_e = np.ascontiguousarray(sw_w2[e * SWC:(e + 1) * SWC, :])
        sw2_p = _pmajor(sw2_e, KCH, f16)                           # (128,12,8)
        ch1_p = _pmajor(
            np.ascontiguousarray(ch_w1[:, e * CH1C:(e + 1) * CH1C]), KCE, f16)
        ch2_p = _pmajor(
            np.ascontiguousarray(ch_w2[e * CH1C:(e + 1) * CH1C, :]), 3, f16)
        oh_p = np.zeros((B, E), dtype=fp32)
        oh_p[:, e] = 1.0
        m = {
            "xp": xp_p, "embq": embq_p, "x2h": x2h, "xft": xft_p,
            "sw1p": sw1_p, "oh": oh_p,
            "w1p": w1_p, "w2p8": w2p8, "w2p16": w2p16,
            "sw2p": sw2_p, "ch1p": ch1_p, "ch2p": ch2_p,
        }
        if include_bias:
            m["b1d"] = np.asarray(inputs["b1"][e], fp32).reshape(1, KD)
            m["b2d"] = np.asarray(inputs["b2"][e], fp32).reshape(1, KD)
            m["swb1d"] = np.asarray(
                inputs["sw_b1"], fp32).reshape(1, ND)[:, e * SWC:(e + 1) * SWC]
            m["swb2d"] = np.asarray(inputs["sw_b2"], fp32).reshape(1, E)
            m["chb1d"] = np.asarray(
                inputs["ch_b1"], fp32).reshape(1, KD)[:, e * CH1C:(e + 1) * CH1C]
        in_maps.append(m)
    return in_maps


def _need_bias(inputs) -> bool:
    return any(
        float(np.abs(np.asarray(inputs[k])).max()) != 0.0
        for k in ("b1", "b2", "sw_b1", "sw_b2", "ch_b1")
    )


def run(inputs: dict, **run_kwargs):
    """Run on the 8 cores; returns (full_output, BassKernelResults)."""
    include_bias = _need_bias(inputs)
    nc = _get_nc(include_bias)
    in_maps = _pack_inputs(inputs, include_bias)
    res = run_bass_kernel_spmd(nc, in_maps, core_ids=list(range(NCORES)), **run_kwargs)
    out = np.zeros((B, C), dtype=np.float64)
    for e in range(NCORES):
        out += res.results[e]["outp"].astype(np.float64)
    out += np.asarray(inputs["ch_b2"], np.float64)
    return out.astype(np.float32), res


def kernel(**inputs) -> np.ndarray:
    out, _ = run(inputs)
    return out
